# revision 48
# baseline (speedup 1.0000x reference)
"""BivectorRotarySelfAttention TRN2 kernel.

Sharding: 8 cores = 4 batches x 2 head-halves. Each core computes one batch's
attention for 8 heads (2 kv heads) and a partial output projection; host sums
the two head-half partials per batch.

Per-core dataflow (transposed layouts: features in partitions, seq in free):
  xT    = recombine(dma_transpose(x_hi), dma_transpose(x_lo))      [f32r]
  qT/kT/vT = W-blocks.T @ xT   (PSUM-accumulated f32r matmuls)
  rope via PE permutation-matmul + 2 DVE muls + 1 add
  scores S^T[m,q]: 4 K=64 matmuls (S0,S1 / C0,C1 row-packed pairs)
  raw = S0*S1 + c'*C0*C1 ; E = exp(alpha*raw + key_mask_bias)  [bf16]
  causal: affine_select on diagonal blocks (GPSIMD)
  outT[d,q] = v-blocks.T @ E (bf16), rowsums via ones-matmul broadcast
  y[l,:] += (outT_h * recip_rowsum) @ Wo_h   (bf16)
"""
import sys
if '/opt/trn_rl_repo' not in sys.path:
    sys.path.insert(0, '/opt/trn_rl_repo')

import numpy as np
import ml_dtypes

import concourse.bass as bass
import concourse.mybir as mybir
import concourse.tile as tile
from concourse import bacc
from concourse.bass_utils import run_bass_kernel_spmd

F32 = mybir.dt.float32
F32R = mybir.dt.float32r
BF16 = mybir.dt.bfloat16

B, L, D, H, HKV = 4, 1024, 2048, 16, 4
HD = D // H            # 128
HD2 = HD // 2          # 64
NH = 8                 # heads per core
NKV = 2                # kv heads per core
NB = L // 128          # 8 blocks of 128
AluOp = mybir.AluOpType
Act = mybir.ActivationFunctionType

_CACHED = {}


def _chunks_for_stripe(mb):
    """Q-column chunks [(qs, qe)] covering [128*mb, 1024), split at 256-multiples."""
    q0 = 128 * mb
    out = []
    while q0 < L:
        qe = min(L, (q0 // 256 + 1) * 256)
        out.append((q0, qe))
        q0 = qe
    return out


def build_program():
    nc = bacc.Bacc("TRN2", target_bir_lowering=False, debug=False)

    # ---- dram params (per-core shapes) ----
    xh = nc.declare_dram_parameter("xh", [L, D], BF16, isOutput=False)
    xl = nc.declare_dram_parameter("xl", [L, D], BF16, isOutput=False)
    wq = nc.declare_dram_parameter("wq", [128, 16, NH * 128], F32R, isOutput=False)
    wk = nc.declare_dram_parameter("wk", [128, 16, NKV * 128], F32R, isOutput=False)
    wv = nc.declare_dram_parameter("wv", [128, 16, NKV * 128], F32R, isOutput=False)
    wo = nc.declare_dram_parameter("wo", [128, NH, D], BF16, isOutput=False)
    cosq = nc.declare_dram_parameter("cosq", [128, NH, L], F32, isOutput=False)
    sinq = nc.declare_dram_parameter("sinq", [128, NH, L], F32, isOutput=False)
    cosk = nc.declare_dram_parameter("cosk", [128, NKV, L], F32, isOutput=False)
    sink = nc.declare_dram_parameter("sink", [128, NKV, L], F32, isOutput=False)
    maskb = nc.declare_dram_parameter("maskb", [128, NB], F32, isOutput=False)
    cprime = nc.declare_dram_parameter("cprime", [128, NH], F32, isOutput=False)
    alpha = nc.declare_dram_parameter("alpha", [128, NH], F32, isOutput=False)
    pmrot = nc.declare_dram_parameter("pmrot", [128, 128], F32R, isOutput=False)
    pmswap = nc.declare_dram_parameter("pmswap", [128, 128], F32R, isOutput=False)
    onesb = nc.declare_dram_parameter("onesb", [128, 128], BF16, isOutput=False)
    identb = nc.declare_dram_parameter("identb", [128, 128], BF16, isOutput=False)
    y = nc.declare_dram_parameter("y", [L, D], F32, isOutput=True)

    with tile.TileContext(nc) as tc:
        with (
            tc.tile_pool(name="persist", bufs=1) as pp,
            tc.tile_pool(name="psum", bufs=1, space="PSUM") as psp,
        ):
            # persistent tiles
            consts = {}
            for nm, src, dt_ in [("pmrot", pmrot, F32R), ("pmswap", pmswap, F32R),
                                 ("onesb", onesb, BF16), ("identb", identb, BF16),
                                 ("maskb", maskb, F32), ("cprime", cprime, F32),
                                 ("alpha", alpha, F32)]:
                t = pp.tile(list(src.shape), dt_, tag=nm, name=nm)
                nc.sync.dma_start(t[:], src[:])
                consts[nm] = t

            xt = [pp.tile([128, L], F32R, tag=f"xt{ib}", name=f"xt{ib}")
                  for ib in range(16)]
            krt = [pp.tile([128, L], F32R, tag=f"krt{g}", name=f"krt{g}")
                   for g in range(NKV)]
            kswap = [pp.tile([128, L], F32R, tag=f"ksw{g}", name=f"ksw{g}")
                     for g in range(NKV)]
            vblk = [pp.tile([128, 128], BF16, tag=f"vb{i}", name=f"vb{i}")
                    for i in range(NKV * NB)]
            outtn = [pp.tile([128, L], BF16, tag=f"ot{h}", name=f"ot{h}")
                     for h in range(NH)]

            # ---------------- prologue: xT + k/v proj + k rope + v transpose
            with tc.tile_pool(name="pro", bufs=1) as ppro:
                # x transpose-load + recombine
                for ib in range(16):
                    th = ppro.tile([128, L], BF16, tag="xh_t", bufs=3)
                    tl = ppro.tile([128, L], BF16, tag="xl_t", bufs=3)
                    nc.sync.dma_start_transpose(th[:], xh[:, ib * 128:(ib + 1) * 128])
                    nc.sync.dma_start_transpose(tl[:], xl[:, ib * 128:(ib + 1) * 128])
                    nc.vector.tensor_add(xt[ib][:], th[:], tl[:])

                wk_t = ppro.tile([128, 16, NKV * 128], F32R, tag="wk")
                wv_t = ppro.tile([128, 16, NKV * 128], F32R, tag="wv")
                nc.sync.dma_start(wk_t[:], wk[:])
                nc.sync.dma_start(wv_t[:], wv[:])

                kt_s = []
                for g in range(NKV):
                    ps = psp.tile([128, L], F32, tag="pj", bufs=1)
                    for ib in range(16):
                        for c in range(2):
                            nc.tensor.matmul(
                                ps[:, c * 512:(c + 1) * 512],
                                wk_t[:, ib, g * 128:(g + 1) * 128],
                                xt[ib][:, c * 512:(c + 1) * 512],
                                start=(ib == 0), stop=(ib == 15))
                    kt = ppro.tile([128, L], F32R, tag="kt_s", bufs=2)
                    nc.any.tensor_copy(kt[:], ps[:])
                    kt_s.append(kt)

                # k rope
                for g in range(NKV):
                    psr = psp.tile([128, L], F32, tag="pj", bufs=1)
                    for c in range(2):
                        nc.tensor.matmul(psr[:, c * 512:(c + 1) * 512],
                                         consts["pmrot"][:],
                                         kt_s[g][:, c * 512:(c + 1) * 512])
                    t1 = ppro.tile([128, L], F32, tag="rtmp", bufs=4)
                    t2 = ppro.tile([128, L], F32, tag="rtmp", bufs=4)
                    csl = ppro.tile([128, L], F32, tag="ktab", bufs=4)
                    snl = ppro.tile([128, L], F32, tag="ktab", bufs=4)
                    nc.sync.dma_start(csl[:], cosk[:, g, :])
                    nc.sync.dma_start(snl[:], sink[:, g, :])
                    nc.vector.tensor_mul(t1[:], psr[:], snl[:])
                    nc.vector.tensor_mul(t2[:], kt_s[g][:].bitcast(F32), csl[:])
                    nc.vector.tensor_add(krt[g][:], t1[:], t2[:])
                    # kswap = partition-swap of krt
                    psw = psp.tile([128, L], F32, tag="pj", bufs=1)
                    for c in range(2):
                        nc.tensor.matmul(psw[:, c * 512:(c + 1) * 512],
                                         consts["pmswap"][:],
                                         krt[g][:, c * 512:(c + 1) * 512])
                    nc.any.tensor_copy(kswap[g][:], psw[:])

                # v proj (bf16 out) + transpose to [m, d] blocks
                for g in range(NKV):
                    ps = psp.tile([128, L], F32, tag="pj", bufs=1)
                    for ib in range(16):
                        for c in range(2):
                            nc.tensor.matmul(
                                ps[:, c * 512:(c + 1) * 512],
                                wv_t[:, ib, g * 128:(g + 1) * 128],
                                xt[ib][:, c * 512:(c + 1) * 512],
                                start=(ib == 0), stop=(ib == 15))
                    vt = ppro.tile([128, L], BF16, tag="vt_s", bufs=2)
                    nc.any.tensor_copy(vt[:], ps[:])
                    for mb in range(NB):
                        pv = psp.tile([128, 128], BF16, tag="pj", bufs=1)
                        nc.tensor.transpose(pv[:], vt[:, mb * 128:(mb + 1) * 128],
                                            consts["identb"][:])
                        nc.vector.tensor_copy(vblk[g * NB + mb][:], pv[:])

            # ---------------- head loop
            with tc.tile_pool(name="hl", bufs=1) as ph:
                for h in range(NH):
                    g = h // 4  # local kv head
                    wq_t = ph.tile([128, 16, 128], F32R, tag="wq_h", bufs=2)
                    nc.sync.dma_start(wq_t[:], wq[:, :, h * 128:(h + 1) * 128])
                    cq = ph.tile([128, L], F32, tag="tabq", bufs=2)
                    sq = ph.tile([128, L], F32, tag="tabq", bufs=2)
                    nc.sync.dma_start(cq[:], cosq[:, h, :])
                    nc.sync.dma_start(sq[:], sinq[:, h, :])

                    psq = psp.tile([128, L], F32, tag="pj", bufs=1)
                    for ib in range(16):
                        for c in range(2):
                            nc.tensor.matmul(
                                psq[:, c * 512:(c + 1) * 512],
                                wq_t[:, ib, :],
                                xt[ib][:, c * 512:(c + 1) * 512],
                                start=(ib == 0), stop=(ib == 15))
                    qt_s = ph.tile([128, L], F32R, tag="qt_s", bufs=2)
                    nc.any.tensor_copy(qt_s[:], psq[:])

                    psr = psp.tile([128, L], F32, tag="pj", bufs=1)
                    for c in range(2):
                        nc.tensor.matmul(psr[:, c * 512:(c + 1) * 512],
                                         consts["pmrot"][:],
                                         qt_s[:, c * 512:(c + 1) * 512])
                    t1 = ph.tile([128, L], F32, tag="qtmp", bufs=2)
                    t2 = ph.tile([128, L], F32, tag="qtmp", bufs=2)
                    nc.vector.tensor_mul(t1[:], psr[:], sq[:])
                    nc.vector.tensor_mul(t2[:], qt_s[:].bitcast(F32), cq[:])
                    qrt = ph.tile([128, L], F32R, tag="qrt", bufs=2)
                    nc.vector.tensor_add(qrt[:], t1[:], t2[:])

                    # scores -> E tiles
                    etiles = []
                    for mb in range(NB):
                        w = L - 128 * mb
                        et = ph.tile([128, w], BF16, tag=f"esc{mb}", bufs=3,
                                     name=f"esc_h{mb}")
                        etiles.append(et)
                    for mb in range(NB):
                        kb = slice(mb * 128, (mb + 1) * 128)
                        for (qs, qe) in _chunks_for_stripe(mb):
                            s = qe - qs
                            psA = psp.tile([128, 2 * s], F32, tag="scA", bufs=1,
                                           name="psA")
                            psB = psp.tile([128, 2 * s], F32, tag="scB", bufs=1,
                                           name="psB")
                            nc.tensor.matmul(psA[:, 0:s], krt[g][0:64, kb],
                                             qrt[0:64, qs:qe])
                            nc.tensor.matmul(psA[:, s:2 * s], kswap[g][0:64, kb],
                                             qrt[0:64, qs:qe])
                            nc.tensor.matmul(psB[:, 0:s], krt[g][64:128, kb],
                                             qrt[64:128, qs:qe])
                            nc.tensor.matmul(psB[:, s:2 * s], kswap[g][64:128, kb],
                                             qrt[64:128, qs:qe])
                            bs = ph.tile([128, 2 * s], F32, tag="bs", bufs=3)
                            nc.any.tensor_copy(bs[:], psB[:])
                            tp = ph.tile([128, 2 * s], F32, tag="tprod", bufs=3)
                            nc.vector.tensor_mul(tp[:], psA[:], bs[:])
                            raw = ph.tile([128, s], F32, tag="raw", bufs=3)
                            nc.vector.scalar_tensor_tensor(
                                raw[:], tp[:, s:2 * s], consts["cprime"][:, h:h + 1],
                                tp[:, 0:s], op0=AluOp.mult, op1=AluOp.add)
                            esl = etiles[mb][:, qs - 128 * mb: qe - 128 * mb]
                            nc.scalar.activation(esl, raw[:], Act.Exp,
                                                 bias=consts["maskb"][:, mb:mb + 1],
                                                 scale=consts["alpha"][:, h:h + 1])
                            if qs == 128 * mb:
                                # causal triangle on the diagonal 128 cols
                                nc.gpsimd.affine_select(
                                    etiles[mb][:, 0:128], etiles[mb][:, 0:128],
                                    pattern=[[1, 128]], compare_op=AluOp.is_ge,
                                    fill=0.0, base=0, channel_multiplier=-1)

                    # attnv + rowsum
                    ps_o = psp.tile([128, L], F32, tag="acco", bufs=1, name="ps_o")
                    ps_rs = psp.tile([128, L], F32, tag="accr", bufs=1, name="ps_rs")
                    for c in range(2):
                        mbs = [mb for mb in range(NB) if 128 * mb < 512 * (c + 1)]
                        for i, mb in enumerate(mbs):
                            os_ = max(512 * c, 128 * mb)
                            oe = 512 * (c + 1)
                            esl = etiles[mb][:, os_ - 128 * mb: oe - 128 * mb]
                            st, sp = (i == 0), (i == len(mbs) - 1)
                            nc.tensor.matmul(ps_o[:, os_:oe], vblk[g * NB + mb][:],
                                             esl, start=st, stop=sp)
                            nc.tensor.matmul(ps_rs[:, os_:oe], consts["onesb"][:],
                                             esl, start=st, stop=sp)
                    rcp = ph.tile([128, L], F32, tag="rcp", bufs=1)
                    nc.vector.reciprocal_approx_fast(rcp[:], ps_rs[:])
                    nc.vector.tensor_mul(outtn[h][:], ps_o[:], rcp[:])

            # ---------------- epilogue: Wo projection
            with tc.tile_pool(name="ep", bufs=1) as pe:
                wo_t = []
                for hb in range(NH):
                    t = pe.tile([128, D], BF16, tag=f"wo{hb}", name=f"wo{hb}")
                    nc.sync.dma_start(t[:], wo[:, hb, :])
                    wo_t.append(t)
                for lb in range(NB):
                    for c in range(2):
                        psy = psp.tile([128, 1024], F32, tag="pj", bufs=1, name="psy")
                        for cc in range(2):
                            for hh in range(NH):
                                nc.tensor.matmul(
                                    psy[:, cc * 512:(cc + 1) * 512],
                                    outtn[hh][:, lb * 128:(lb + 1) * 128],
                                    wo_t[hh][:, c * 1024 + cc * 512:
                                            c * 1024 + (cc + 1) * 512],
                                    start=(hh == 0), stop=(hh == NH - 1))
                        yt = pe.tile([128, 1024], F32, tag="ytile", bufs=3)
                        nc.any.tensor_copy(yt[:], psy[:])
                        nc.sync.dma_start(
                            y[lb * 128:(lb + 1) * 128, c * 1024:(c + 1) * 1024], yt[:])

    nc.compile()
    return nc


def _host_prep(x, Wq, Wk, Wv, Wo, q_param, log_scale, cos, sin, mask):
    """Build the 8 per-core input maps."""
    x = np.asarray(x, np.float32)
    Wq = np.asarray(Wq, np.float32)
    Wk = np.asarray(Wk, np.float32)
    Wv = np.asarray(Wv, np.float32)
    Wo = np.asarray(Wo, np.float32)
    cos = np.asarray(cos, np.float32)[0]      # [L, H, 64]
    sin = np.asarray(sin, np.float32)[0]
    qp = np.asarray(q_param, np.float32).reshape(H)
    ls = np.asarray(log_scale, np.float32).reshape(H)
    mask = np.asarray(mask)

    p64 = np.arange(128) % 64

    PM = np.zeros((128, 128), np.float32)
    for dp in range(128):
        base, r = (dp // 64) * 64, dp % 64
        if r < 32:
            PM[base + r + 32, dp] = -1.0
        else:
            PM[base + r - 32, dp] = 1.0
    SW = np.zeros((128, 128), np.float32)
    for dp in range(128):
        SW[(dp + 64) % 128, dp] = 1.0
    ONES = np.ones((128, 128), ml_dtypes.bfloat16)
    IDENT = np.eye(128, dtype=ml_dtypes.bfloat16)

    in_maps = []
    for core in range(8):
        b, g2 = core // 2, core % 2
        heads = list(range(g2 * NH, (g2 + 1) * NH))
        kvs = list(range(g2 * NKV, (g2 + 1) * NKV))

        xb = x[b]
        xh = xb.astype(ml_dtypes.bfloat16)
        xlo = (xb - xh.astype(np.float32)).astype(ml_dtypes.bfloat16)

        wq_c = Wq[:, g2 * NH * 128:(g2 + 1) * NH * 128]
        wk_c = Wk[:, g2 * NKV * 128:(g2 + 1) * NKV * 128]
        wv_c = Wv[:, g2 * NKV * 128:(g2 + 1) * NKV * 128]
        wo_c = Wo[g2 * NH * 128:(g2 + 1) * NH * 128, :]

        wq_p = wq_c.reshape(16, 128, NH * 128).transpose(1, 0, 2).copy()
        wk_p = wk_c.reshape(16, 128, NKV * 128).transpose(1, 0, 2).copy()
        wv_p = wv_c.reshape(16, 128, NKV * 128).transpose(1, 0, 2).copy()
        wo_p = wo_c.reshape(NH, 128, D).transpose(1, 0, 2).astype(ml_dtypes.bfloat16)

        cosq_p = np.ascontiguousarray(cos[:, heads, :][:, :, p64].transpose(2, 1, 0))
        sinq_p = np.ascontiguousarray(sin[:, heads, :][:, :, p64].transpose(2, 1, 0))
        cosk_p = np.ascontiguousarray(cos[:, kvs, :][:, :, p64].transpose(2, 1, 0))
        sink_p = np.ascontiguousarray(sin[:, kvs, :][:, :, p64].transpose(2, 1, 0))

        mb = np.where(mask[b].reshape(NB, 128).T.astype(bool), 0.0, -1e9)
        mb = mb.astype(np.float32)

        cpr = np.tile((-2.0 * np.tanh(qp[heads]))[None, :], (128, 1))
        alp = np.tile((np.exp(ls[heads]) / HD)[None, :], (128, 1))

        in_maps.append({
            "xh": xh, "xl": xlo,
            "wq": wq_p.astype(np.float32), "wk": wk_p.astype(np.float32),
            "wv": wv_p.astype(np.float32), "wo": wo_p,
            "cosq": cosq_p, "sinq": sinq_p, "cosk": cosk_p, "sink": sink_p,
            "maskb": mb, "cprime": cpr.astype(np.float32),
            "alpha": alp.astype(np.float32),
            "pmrot": PM, "pmswap": SW, "onesb": ONES, "identb": IDENT,
        })
    return in_maps


def kernel(**inputs):
    if "nc" not in _CACHED:
        _CACHED["nc"] = build_program()
    nc = _CACHED["nc"]
    in_maps = _host_prep(**inputs)
    res = run_bass_kernel_spmd(nc, in_maps, list(range(8))).results
    out = np.empty((B, L, D), np.float32)
    for b in range(B):
        out[b] = res[2 * b]["y"] + res[2 * b + 1]["y"]
    return out


# revision 54
# speedup vs baseline: 1.2629x; 1.2629x over previous
"""BivectorRotarySelfAttention TRN2 kernel.

Sharding: 8 cores = 4 batches x 2 head-halves. Each core computes one batch's
attention for 8 heads (2 kv heads) and a partial output projection; host sums
the two head-half partials per batch.

Per-core dataflow (transposed layouts: features in partitions, seq in free):
  xT    = recombine(dma_transpose(x_hi), dma_transpose(x_lo))      [f32r]
  qT/kT/vT = W-blocks.T @ xT   (PSUM-accumulated f32r matmuls)
  rope via PE permutation-matmul + 2 DVE muls + 1 add
  scores S^T[m,q]: 4 K=64 matmuls (S0,S1 / C0,C1 row-packed pairs)
  raw = S0*S1 + c'*C0*C1 ; E = exp(alpha*raw + key_mask_bias)  [bf16]
  causal: affine_select on diagonal blocks (GPSIMD)
  outT[d,q] = v-blocks.T @ E (bf16), rowsums via ones-matmul broadcast
  y[l,:] += (outT_h * recip_rowsum) @ Wo_h   (bf16)
"""
import sys
if '/opt/trn_rl_repo' not in sys.path:
    sys.path.insert(0, '/opt/trn_rl_repo')

import numpy as np
import ml_dtypes

import concourse.bass as bass
import concourse.mybir as mybir
import concourse.tile as tile
from concourse import bacc
from concourse.bass_utils import run_bass_kernel_spmd

F32 = mybir.dt.float32
F32R = mybir.dt.float32r
BF16 = mybir.dt.bfloat16

B, L, D, H, HKV = 4, 1024, 2048, 16, 4
HD = D // H            # 128
HD2 = HD // 2          # 64
NH = 8                 # heads per core
NKV = 2                # kv heads per core
NB = L // 128          # 8 blocks of 128
AluOp = mybir.AluOpType
Act = mybir.ActivationFunctionType

_CACHED = {}


def _chunks_for_stripe(mb):
    """Q-column chunks [(qs, qe)] covering [128*mb, 1024), split at 256-multiples."""
    q0 = 128 * mb
    out = []
    while q0 < L:
        qe = min(L, (q0 // 256 + 1) * 256)
        out.append((q0, qe))
        q0 = qe
    return out


def build_program():
    nc = bacc.Bacc("TRN2", target_bir_lowering=False, debug=False)

    # ---- dram params (per-core shapes) ----
    xh = nc.declare_dram_parameter("xh", [L, D], BF16, isOutput=False)
    xl = nc.declare_dram_parameter("xl", [L, D], BF16, isOutput=False)
    wq = nc.declare_dram_parameter("wq", [128, 16, NH * 128], F32R, isOutput=False)
    wk = nc.declare_dram_parameter("wk", [128, 16, NKV * 128], F32R, isOutput=False)
    wv = nc.declare_dram_parameter("wv", [128, 16, NKV * 128], F32R, isOutput=False)
    wo = nc.declare_dram_parameter("wo", [128, NH, D], BF16, isOutput=False)
    cosq = nc.declare_dram_parameter("cosq", [128, NH, L], BF16, isOutput=False)
    sinq = nc.declare_dram_parameter("sinq", [128, NH, L], BF16, isOutput=False)
    cosk = nc.declare_dram_parameter("cosk", [128, NKV, L], BF16, isOutput=False)
    sink = nc.declare_dram_parameter("sink", [128, NKV, L], BF16, isOutput=False)
    maskb = nc.declare_dram_parameter("maskb", [128, NB], F32, isOutput=False)
    cprime = nc.declare_dram_parameter("cprime", [128, NH], F32, isOutput=False)
    alpha = nc.declare_dram_parameter("alpha", [128, NH], F32, isOutput=False)
    pmrot = nc.declare_dram_parameter("pmrot", [128, 128], BF16, isOutput=False)
    pmswap = nc.declare_dram_parameter("pmswap", [128, 128], BF16, isOutput=False)
    onesb = nc.declare_dram_parameter("onesb", [128, 128], BF16, isOutput=False)
    identb = nc.declare_dram_parameter("identb", [128, 128], BF16, isOutput=False)
    y = nc.declare_dram_parameter("y", [L, D], BF16, isOutput=True)

    with tile.TileContext(nc) as tc:
        with (
            tc.tile_pool(name="persist", bufs=1) as pp,
            tc.tile_pool(name="psum", bufs=1, space="PSUM") as psp,
        ):
            # persistent tiles
            consts = {}
            for nm, src, dt_ in [("pmrot", pmrot, BF16), ("pmswap", pmswap, BF16),
                                 ("onesb", onesb, BF16), ("identb", identb, BF16),
                                 ("maskb", maskb, F32), ("cprime", cprime, F32),
                                 ("alpha", alpha, F32)]:
                t = pp.tile(list(src.shape), dt_, tag=nm, name=nm)
                nc.scalar.dma_start(t[:], src[:])
                consts[nm] = t

            xt = [pp.tile([128, L], F32R, tag=f"xt{ib}", name=f"xt{ib}")
                  for ib in range(16)]
            krt = [pp.tile([128, L], BF16, tag=f"krt{g}", name=f"krt{g}")
                   for g in range(NKV)]
            kswap = [pp.tile([128, L], BF16, tag=f"ksw{g}", name=f"ksw{g}")
                     for g in range(NKV)]
            vblk = [pp.tile([128, 128], BF16, tag=f"vb{i}", name=f"vb{i}")
                    for i in range(NKV * NB)]
            outtn = [pp.tile([128, L], BF16, tag=f"ot{h}", name=f"ot{h}")
                     for h in range(NH)]
            wo_t = [pp.tile([128, D], BF16, tag=f"wo{h}", name=f"wo{h}")
                    for h in range(NH)]

            # ---------------- prologue: xT + k/v proj + k rope + v transpose
            with tc.tile_pool(name="pro", bufs=1) as ppro:
                # x transpose-load + recombine
                for ib in range(16):
                    th = ppro.tile([128, L], BF16, tag="xh_t", bufs=3)
                    tl = ppro.tile([128, L], BF16, tag="xl_t", bufs=3)
                    nc.sync.dma_start_transpose(th[:], xh[:, ib * 128:(ib + 1) * 128])
                    nc.sync.dma_start_transpose(tl[:], xl[:, ib * 128:(ib + 1) * 128])
                    nc.vector.tensor_add(xt[ib][:], th[:], tl[:])

                wk_t = ppro.tile([128, 16, NKV * 128], F32R, tag="wk")
                wv_t = ppro.tile([128, 16, NKV * 128], F32R, tag="wv")
                nc.scalar.dma_start(wk_t[:], wk[:])
                nc.scalar.dma_start(wv_t[:], wv[:])

                kt_s = []
                for g in range(NKV):
                    ps = psp.tile([128, L], F32, tag="pj", bufs=1)
                    for ib in range(16):
                        for c in range(2):
                            nc.tensor.matmul(
                                ps[:, c * 512:(c + 1) * 512],
                                wk_t[:, ib, g * 128:(g + 1) * 128],
                                xt[ib][:, c * 512:(c + 1) * 512],
                                start=(ib == 0), stop=(ib == 15))
                    kt = ppro.tile([128, L], BF16, tag="kt_s", bufs=2)
                    nc.any.tensor_copy(kt[:], ps[:])
                    kt_s.append(kt)

                # k rope
                for g in range(NKV):
                    psr = psp.tile([128, L], F32, tag="pj", bufs=1)
                    for c in range(2):
                        nc.tensor.matmul(psr[:, c * 512:(c + 1) * 512],
                                         consts["pmrot"][:],
                                         kt_s[g][:, c * 512:(c + 1) * 512])
                    t1 = ppro.tile([128, L], BF16, tag="rtmp", bufs=4)
                    t2 = ppro.tile([128, L], BF16, tag="rtmp", bufs=4)
                    csl = ppro.tile([128, L], BF16, tag="ktab", bufs=4)
                    snl = ppro.tile([128, L], BF16, tag="ktab", bufs=4)
                    nc.sync.dma_start(csl[:], cosk[:, g, :])
                    nc.sync.dma_start(snl[:], sink[:, g, :])
                    nc.vector.tensor_mul(t1[:], psr[:], snl[:])
                    nc.vector.tensor_mul(t2[:], kt_s[g][:], csl[:])
                    nc.vector.tensor_add(krt[g][:], t1[:], t2[:])
                    # kswap = partition-swap of krt
                    psw = psp.tile([128, L], F32, tag="pj", bufs=1)
                    for c in range(2):
                        nc.tensor.matmul(psw[:, c * 512:(c + 1) * 512],
                                         consts["pmswap"][:],
                                         krt[g][:, c * 512:(c + 1) * 512])
                    nc.any.tensor_copy(kswap[g][:], psw[:])

                # v proj (bf16 out) + transpose to [m, d] blocks
                for g in range(NKV):
                    ps = psp.tile([128, L], F32, tag="pj", bufs=1)
                    for ib in range(16):
                        for c in range(2):
                            nc.tensor.matmul(
                                ps[:, c * 512:(c + 1) * 512],
                                wv_t[:, ib, g * 128:(g + 1) * 128],
                                xt[ib][:, c * 512:(c + 1) * 512],
                                start=(ib == 0), stop=(ib == 15))
                    vt = ppro.tile([128, L], BF16, tag="vt_s", bufs=2)
                    nc.any.tensor_copy(vt[:], ps[:])
                    for mb in range(NB):
                        pv = psp.tile([128, 128], BF16, tag="pj", bufs=1)
                        nc.tensor.transpose(pv[:], vt[:, mb * 128:(mb + 1) * 128],
                                            consts["identb"][:])
                        nc.vector.tensor_copy(vblk[g * NB + mb][:], pv[:])

            # ---------------- head loop
            with tc.tile_pool(name="hl", bufs=1) as ph:
                for h in range(NH):
                    g = h // 4  # local kv head
                    wq_t = ph.tile([128, 16, 128], F32R, tag="wq_h", bufs=2)
                    nc.sync.dma_start(wq_t[:], wq[:, :, h * 128:(h + 1) * 128])
                    cq = ph.tile([128, L], F32, tag="tabq", bufs=2)
                    sq = ph.tile([128, L], F32, tag="tabq", bufs=2)
                    nc.sync.dma_start(cq[:], cosq[:, h, :])
                    nc.sync.dma_start(sq[:], sinq[:, h, :])

                    psq = psp.tile([128, L], F32, tag="pj", bufs=1)
                    for ib in range(16):
                        for c in range(2):
                            nc.tensor.matmul(
                                psq[:, c * 512:(c + 1) * 512],
                                wq_t[:, ib, :],
                                xt[ib][:, c * 512:(c + 1) * 512],
                                start=(ib == 0), stop=(ib == 15))
                    qt_s = ph.tile([128, L], F32R, tag="qt_s", bufs=2)
                    nc.any.tensor_copy(qt_s[:], psq[:])

                    psr = psp.tile([128, L], F32, tag="pj", bufs=1)
                    for c in range(2):
                        nc.tensor.matmul(psr[:, c * 512:(c + 1) * 512],
                                         consts["pmrot"][:],
                                         qt_s[:, c * 512:(c + 1) * 512])
                    t1 = ph.tile([128, L], F32, tag="qtmp", bufs=2)
                    t2 = ph.tile([128, L], F32, tag="qtmp", bufs=2)
                    nc.vector.tensor_mul(t1[:], psr[:], sq[:])
                    nc.vector.tensor_mul(t2[:], qt_s[:].bitcast(F32), cq[:])
                    qrt = ph.tile([128, L], F32R, tag="qrt", bufs=2)
                    nc.vector.tensor_add(qrt[:], t1[:], t2[:])

                    # scores -> E tiles
                    etiles = []
                    for mb in range(NB):
                        w = L - 128 * mb
                        et = ph.tile([128, w], BF16, tag=f"esc{mb}", bufs=3,
                                     name=f"esc_h{mb}")
                        etiles.append(et)
                    for mb in range(NB):
                        kb = slice(mb * 128, (mb + 1) * 128)
                        for (qs, qe) in _chunks_for_stripe(mb):
                            s = qe - qs
                            psA = psp.tile([128, 2 * s], F32, tag="scA", bufs=1,
                                           name="psA")
                            psB = psp.tile([128, 2 * s], F32, tag="scB", bufs=1,
                                           name="psB")
                            nc.tensor.matmul(psA[:, 0:s], krt[g][0:64, kb],
                                             qrt[0:64, qs:qe])
                            nc.tensor.matmul(psA[:, s:2 * s], kswap[g][0:64, kb],
                                             qrt[0:64, qs:qe])
                            nc.tensor.matmul(psB[:, 0:s], krt[g][64:128, kb],
                                             qrt[64:128, qs:qe])
                            nc.tensor.matmul(psB[:, s:2 * s], kswap[g][64:128, kb],
                                             qrt[64:128, qs:qe])
                            bs = ph.tile([128, 2 * s], F32, tag="bs", bufs=3)
                            nc.any.tensor_copy(bs[:], psB[:])
                            tp = ph.tile([128, 2 * s], F32, tag="tprod", bufs=3)
                            nc.vector.tensor_mul(tp[:], psA[:], bs[:])
                            raw = ph.tile([128, s], F32, tag="raw", bufs=3)
                            nc.vector.scalar_tensor_tensor(
                                raw[:], tp[:, s:2 * s], consts["cprime"][:, h:h + 1],
                                tp[:, 0:s], op0=AluOp.mult, op1=AluOp.add)
                            esl = etiles[mb][:, qs - 128 * mb: qe - 128 * mb]
                            nc.scalar.activation(esl, raw[:], Act.Exp,
                                                 bias=consts["maskb"][:, mb:mb + 1],
                                                 scale=consts["alpha"][:, h:h + 1])
                            if qs == 128 * mb:
                                # causal triangle on the diagonal 128 cols
                                nc.gpsimd.affine_select(
                                    etiles[mb][:, 0:128], etiles[mb][:, 0:128],
                                    pattern=[[1, 128]], compare_op=AluOp.is_ge,
                                    fill=0.0, base=0, channel_multiplier=-1)

                    # attnv + rowsum
                    ps_o = psp.tile([128, L], F32, tag="acco", bufs=1, name="ps_o")
                    ps_rs = psp.tile([128, L], F32, tag="accr", bufs=1, name="ps_rs")
                    for c in range(2):
                        mbs = [mb for mb in range(NB) if 128 * mb < 512 * (c + 1)]
                        for i, mb in enumerate(mbs):
                            os_ = max(512 * c, 128 * mb)
                            oe = 512 * (c + 1)
                            esl = etiles[mb][:, os_ - 128 * mb: oe - 128 * mb]
                            st, sp = (i == 0), (i == len(mbs) - 1)
                            nc.tensor.matmul(ps_o[:, os_:oe], vblk[g * NB + mb][:],
                                             esl, start=st, stop=sp)
                            nc.tensor.matmul(ps_rs[:, os_:oe], consts["onesb"][:],
                                             esl, start=st, stop=sp)
                    rcp = ph.tile([128, L], F32, tag="rcp", bufs=1)
                    nc.vector.reciprocal_approx_fast(rcp[:], ps_rs[:])
                    nc.vector.tensor_mul(outtn[h][:], ps_o[:], rcp[:])

            # ---------------- epilogue: Wo projection
            with tc.tile_pool(name="ep", bufs=1) as pe:
                for lb in range(NB):
                    for c in range(2):
                        psy = psp.tile([128, 1024], F32, tag="pj", bufs=1, name="psy")
                        for cc in range(2):
                            for hh in range(NH):
                                nc.tensor.matmul(
                                    psy[:, cc * 512:(cc + 1) * 512],
                                    outtn[hh][:, lb * 128:(lb + 1) * 128],
                                    wo_t[hh][:, c * 1024 + cc * 512:
                                            c * 1024 + (cc + 1) * 512],
                                    start=(hh == 0), stop=(hh == NH - 1))
                        yt = pe.tile([128, 1024], BF16, tag="ytile", bufs=3)
                        nc.any.tensor_copy(yt[:], psy[:])
                        nc.sync.dma_start(
                            y[lb * 128:(lb + 1) * 128, c * 1024:(c + 1) * 1024], yt[:])

    nc.compile()
    return nc


def _host_prep(x, Wq, Wk, Wv, Wo, q_param, log_scale, cos, sin, mask):
    """Build the 8 per-core input maps."""
    x = np.asarray(x, np.float32)
    Wq = np.asarray(Wq, np.float32)
    Wk = np.asarray(Wk, np.float32)
    Wv = np.asarray(Wv, np.float32)
    Wo = np.asarray(Wo, np.float32)
    cos = np.asarray(cos, np.float32)[0]      # [L, H, 64]
    sin = np.asarray(sin, np.float32)[0]
    qp = np.asarray(q_param, np.float32).reshape(H)
    ls = np.asarray(log_scale, np.float32).reshape(H)
    mask = np.asarray(mask)

    p64 = np.arange(128) % 64

    PM = np.zeros((128, 128), np.float32)
    for dp in range(128):
        base, r = (dp // 64) * 64, dp % 64
        if r < 32:
            PM[base + r + 32, dp] = -1.0
        else:
            PM[base + r - 32, dp] = 1.0
    SW = np.zeros((128, 128), np.float32)
    for dp in range(128):
        SW[(dp + 64) % 128, dp] = 1.0
    ONES = np.ones((128, 128), ml_dtypes.bfloat16)
    IDENT = np.eye(128, dtype=ml_dtypes.bfloat16)

    in_maps = []
    for core in range(8):
        b, g2 = core // 2, core % 2
        heads = list(range(g2 * NH, (g2 + 1) * NH))
        kvs = list(range(g2 * NKV, (g2 + 1) * NKV))

        xb = x[b]
        xh = xb.astype(ml_dtypes.bfloat16)
        xlo = (xb - xh.astype(np.float32)).astype(ml_dtypes.bfloat16)

        wq_c = Wq[:, g2 * NH * 128:(g2 + 1) * NH * 128]
        wk_c = Wk[:, g2 * NKV * 128:(g2 + 1) * NKV * 128]
        wv_c = Wv[:, g2 * NKV * 128:(g2 + 1) * NKV * 128]
        wo_c = Wo[g2 * NH * 128:(g2 + 1) * NH * 128, :]

        wq_p = wq_c.reshape(16, 128, NH * 128).transpose(1, 0, 2).copy()
        wk_p = wk_c.reshape(16, 128, NKV * 128).transpose(1, 0, 2).copy()
        wv_p = wv_c.reshape(16, 128, NKV * 128).transpose(1, 0, 2).copy()
        wo_p = wo_c.reshape(NH, 128, D).transpose(1, 0, 2).astype(ml_dtypes.bfloat16)

        bf = ml_dtypes.bfloat16
        cosq_p = np.ascontiguousarray(
            cos[:, heads, :][:, :, p64].transpose(2, 1, 0)).astype(bf)
        sinq_p = np.ascontiguousarray(
            sin[:, heads, :][:, :, p64].transpose(2, 1, 0)).astype(bf)
        cosk_p = np.ascontiguousarray(
            cos[:, kvs, :][:, :, p64].transpose(2, 1, 0)).astype(bf)
        sink_p = np.ascontiguousarray(
            sin[:, kvs, :][:, :, p64].transpose(2, 1, 0)).astype(bf)

        mb = np.where(mask[b].reshape(NB, 128).T.astype(bool), 0.0, -1e9)
        mb = mb.astype(np.float32)

        cpr = np.tile((-2.0 * np.tanh(qp[heads]))[None, :], (128, 1))
        alp = np.tile((np.exp(ls[heads]) / HD)[None, :], (128, 1))

        in_maps.append({
            "xh": xh, "xl": xlo,
            "wq": wq_p.astype(np.float32), "wk": wk_p.astype(np.float32),
            "wv": wv_p.astype(np.float32), "wo": wo_p,
            "cosq": cosq_p, "sinq": sinq_p, "cosk": cosk_p, "sink": sink_p,
            "maskb": mb, "cprime": cpr.astype(np.float32),
            "alpha": alp.astype(np.float32),
            "pmrot": PM.astype(ml_dtypes.bfloat16),
            "pmswap": SW.astype(ml_dtypes.bfloat16),
            "onesb": ONES, "identb": IDENT,
        })
    return in_maps


def kernel(**inputs):
    if "nc" not in _CACHED:
        _CACHED["nc"] = build_program()
    nc = _CACHED["nc"]
    in_maps = _host_prep(**inputs)
    res = run_bass_kernel_spmd(nc, in_maps, list(range(8))).results
    out = np.empty((B, L, D), np.float32)
    for b in range(B):
        out[b] = (res[2 * b]["y"].astype(np.float32)
                  + res[2 * b + 1]["y"].astype(np.float32))
    return out


# revision 55
# speedup vs baseline: 1.3276x; 1.0513x over previous
"""BivectorRotarySelfAttention TRN2 kernel.

Sharding: 8 cores = 4 batches x 2 head-halves. Each core computes one batch's
attention for 8 heads (2 kv heads) and a partial output projection; host sums
the two head-half partials per batch.

Per-core dataflow (transposed layouts: features in partitions, seq in free):
  xT    = recombine(dma_transpose(x_hi), dma_transpose(x_lo))      [f32r]
  qT/kT/vT = W-blocks.T @ xT   (PSUM-accumulated f32r matmuls)
  rope via PE permutation-matmul + 2 DVE muls + 1 add
  scores S^T[m,q]: 4 K=64 matmuls (S0,S1 / C0,C1 row-packed pairs)
  raw = S0*S1 + c'*C0*C1 ; E = exp(alpha*raw + key_mask_bias)  [bf16]
  causal: affine_select on diagonal blocks (GPSIMD)
  outT[d,q] = v-blocks.T @ E (bf16), rowsums via ones-matmul broadcast
  y[l,:] += (outT_h * recip_rowsum) @ Wo_h   (bf16)
"""
import sys
if '/opt/trn_rl_repo' not in sys.path:
    sys.path.insert(0, '/opt/trn_rl_repo')

import numpy as np
import ml_dtypes

import concourse.bass as bass
import concourse.mybir as mybir
import concourse.tile as tile
from concourse import bacc
from concourse.bass_utils import run_bass_kernel_spmd

F32 = mybir.dt.float32
F32R = mybir.dt.float32r
BF16 = mybir.dt.bfloat16

B, L, D, H, HKV = 4, 1024, 2048, 16, 4
HD = D // H            # 128
HD2 = HD // 2          # 64
NH = 8                 # heads per core
NKV = 2                # kv heads per core
NB = L // 128          # 8 blocks of 128
AluOp = mybir.AluOpType
Act = mybir.ActivationFunctionType

_CACHED = {}


def _chunks_for_stripe(mb):
    """Q-column chunks [(qs, qe)] covering [128*mb, 1024), split at 256-multiples."""
    q0 = 128 * mb
    out = []
    while q0 < L:
        qe = min(L, (q0 // 256 + 1) * 256)
        out.append((q0, qe))
        q0 = qe
    return out


def build_program():
    nc = bacc.Bacc("TRN2", target_bir_lowering=False, debug=False)

    # ---- dram params (per-core shapes) ----
    xh = nc.declare_dram_parameter("xh", [L, D], BF16, isOutput=False)
    xl = nc.declare_dram_parameter("xl", [L, D], BF16, isOutput=False)
    wq = nc.declare_dram_parameter("wq", [128, 16, NH * 128], F32R, isOutput=False)
    wk = nc.declare_dram_parameter("wk", [128, 16, NKV * 128], F32R, isOutput=False)
    wv = nc.declare_dram_parameter("wv", [128, 16, NKV * 128], F32R, isOutput=False)
    wo = nc.declare_dram_parameter("wo", [128, NH, D], BF16, isOutput=False)
    cosq = nc.declare_dram_parameter("cosq", [128, NH, L], BF16, isOutput=False)
    sinq = nc.declare_dram_parameter("sinq", [128, NH, L], BF16, isOutput=False)
    cosk = nc.declare_dram_parameter("cosk", [128, NKV, L], BF16, isOutput=False)
    sink = nc.declare_dram_parameter("sink", [128, NKV, L], BF16, isOutput=False)
    maskb = nc.declare_dram_parameter("maskb", [128, NB], F32, isOutput=False)
    cprime = nc.declare_dram_parameter("cprime", [128, NH], F32, isOutput=False)
    alpha = nc.declare_dram_parameter("alpha", [128, NH], F32, isOutput=False)
    pmrot = nc.declare_dram_parameter("pmrot", [128, 128], BF16, isOutput=False)
    pmswap = nc.declare_dram_parameter("pmswap", [128, 128], BF16, isOutput=False)
    onesb = nc.declare_dram_parameter("onesb", [128, 128], BF16, isOutput=False)
    identb = nc.declare_dram_parameter("identb", [128, 128], BF16, isOutput=False)
    y = nc.declare_dram_parameter("y", [L, D], BF16, isOutput=True)

    with tile.TileContext(nc) as tc:
        with (
            tc.tile_pool(name="persist", bufs=1) as pp,
            tc.tile_pool(name="psum", bufs=1, space="PSUM") as psp,
        ):
            # persistent tiles
            consts = {}
            for nm, src, dt_ in [("pmrot", pmrot, BF16), ("pmswap", pmswap, BF16),
                                 ("onesb", onesb, BF16), ("identb", identb, BF16),
                                 ("maskb", maskb, F32), ("cprime", cprime, F32),
                                 ("alpha", alpha, F32)]:
                t = pp.tile(list(src.shape), dt_, tag=nm, name=nm)
                nc.sync.dma_start(t[:], src[:])
                consts[nm] = t

            xt = [pp.tile([128, L], F32R, tag=f"xt{ib}", name=f"xt{ib}")
                  for ib in range(16)]
            krt = [pp.tile([128, L], BF16, tag=f"krt{g}", name=f"krt{g}")
                   for g in range(NKV)]
            kswap = [pp.tile([128, L], BF16, tag=f"ksw{g}", name=f"ksw{g}")
                     for g in range(NKV)]
            vblk = [pp.tile([128, 128], BF16, tag=f"vb{i}", name=f"vb{i}")
                    for i in range(NKV * NB)]
            outtn = [pp.tile([128, L], BF16, tag=f"ot{h}", name=f"ot{h}")
                     for h in range(NH)]
            wo_t = [pp.tile([128, D], BF16, tag=f"wo{h}", name=f"wo{h}")
                    for h in range(NH)]

            # ---------------- prologue: xT + k/v proj + k rope + v transpose
            with tc.tile_pool(name="pro", bufs=1) as ppro:
                # x transpose-load + recombine
                for ib in range(16):
                    th = ppro.tile([128, L], BF16, tag="xh_t", bufs=3)
                    tl = ppro.tile([128, L], BF16, tag="xl_t", bufs=3)
                    nc.sync.dma_start_transpose(th[:], xh[:, ib * 128:(ib + 1) * 128])
                    nc.sync.dma_start_transpose(tl[:], xl[:, ib * 128:(ib + 1) * 128])
                    nc.vector.tensor_add(xt[ib][:], th[:], tl[:])

                wk_t = ppro.tile([128, 16, NKV * 128], F32R, tag="wk")
                wv_t = ppro.tile([128, 16, NKV * 128], F32R, tag="wv")
                nc.scalar.dma_start(wk_t[:], wk[:])
                nc.scalar.dma_start(wv_t[:], wv[:])

                kt_s = []
                for g in range(NKV):
                    ps = psp.tile([128, L], F32, tag="pj", bufs=1)
                    for ib in range(16):
                        for c in range(2):
                            nc.tensor.matmul(
                                ps[:, c * 512:(c + 1) * 512],
                                wk_t[:, ib, g * 128:(g + 1) * 128],
                                xt[ib][:, c * 512:(c + 1) * 512],
                                start=(ib == 0), stop=(ib == 15))
                    kt = ppro.tile([128, L], BF16, tag="kt_s", bufs=2)
                    nc.any.tensor_copy(kt[:], ps[:])
                    kt_s.append(kt)

                # k rope
                for g in range(NKV):
                    psr = psp.tile([128, L], F32, tag="pj", bufs=1)
                    for c in range(2):
                        nc.tensor.matmul(psr[:, c * 512:(c + 1) * 512],
                                         consts["pmrot"][:],
                                         kt_s[g][:, c * 512:(c + 1) * 512])
                    t1 = ppro.tile([128, L], BF16, tag="rtmp", bufs=4)
                    t2 = ppro.tile([128, L], BF16, tag="rtmp", bufs=4)
                    csl = ppro.tile([128, L], BF16, tag="ktab", bufs=4)
                    snl = ppro.tile([128, L], BF16, tag="ktab", bufs=4)
                    nc.sync.dma_start(csl[:], cosk[:, g, :])
                    nc.sync.dma_start(snl[:], sink[:, g, :])
                    nc.vector.tensor_mul(t1[:], psr[:], snl[:])
                    nc.vector.tensor_mul(t2[:], kt_s[g][:], csl[:])
                    nc.vector.tensor_add(krt[g][:], t1[:], t2[:])
                    # kswap = partition-swap of krt
                    psw = psp.tile([128, L], F32, tag="pj", bufs=1)
                    for c in range(2):
                        nc.tensor.matmul(psw[:, c * 512:(c + 1) * 512],
                                         consts["pmswap"][:],
                                         krt[g][:, c * 512:(c + 1) * 512])
                    nc.any.tensor_copy(kswap[g][:], psw[:])

                # v proj (bf16 out) + transpose to [m, d] blocks
                for g in range(NKV):
                    ps = psp.tile([128, L], F32, tag="pj", bufs=1)
                    for ib in range(16):
                        for c in range(2):
                            nc.tensor.matmul(
                                ps[:, c * 512:(c + 1) * 512],
                                wv_t[:, ib, g * 128:(g + 1) * 128],
                                xt[ib][:, c * 512:(c + 1) * 512],
                                start=(ib == 0), stop=(ib == 15))
                    vt = ppro.tile([128, L], BF16, tag="vt_s", bufs=2)
                    nc.any.tensor_copy(vt[:], ps[:])
                    for mb in range(NB):
                        pv = psp.tile([128, 128], BF16, tag="pj", bufs=1)
                        nc.tensor.transpose(pv[:], vt[:, mb * 128:(mb + 1) * 128],
                                            consts["identb"][:])
                        nc.vector.tensor_copy(vblk[g * NB + mb][:], pv[:])

            # ---------------- head loop
            with tc.tile_pool(name="hl", bufs=1) as ph:
                for h in range(NH):
                    g = h // 4  # local kv head
                    wq_t = ph.tile([128, 16, 128], F32R, tag="wq_h", bufs=2)
                    nc.sync.dma_start(wq_t[:], wq[:, :, h * 128:(h + 1) * 128])
                    cq = ph.tile([128, L], F32, tag="tabq", bufs=2)
                    sq = ph.tile([128, L], F32, tag="tabq", bufs=2)
                    nc.sync.dma_start(cq[:], cosq[:, h, :])
                    nc.sync.dma_start(sq[:], sinq[:, h, :])

                    psq = psp.tile([128, L], F32, tag="pj", bufs=1)
                    for ib in range(16):
                        for c in range(2):
                            nc.tensor.matmul(
                                psq[:, c * 512:(c + 1) * 512],
                                wq_t[:, ib, :],
                                xt[ib][:, c * 512:(c + 1) * 512],
                                start=(ib == 0), stop=(ib == 15))
                    qt_s = ph.tile([128, L], F32R, tag="qt_s", bufs=2)
                    nc.any.tensor_copy(qt_s[:], psq[:])

                    psr = psp.tile([128, L], F32, tag="pj", bufs=1)
                    for c in range(2):
                        nc.tensor.matmul(psr[:, c * 512:(c + 1) * 512],
                                         consts["pmrot"][:],
                                         qt_s[:, c * 512:(c + 1) * 512])
                    t1 = ph.tile([128, L], F32, tag="qtmp", bufs=2)
                    t2 = ph.tile([128, L], F32, tag="qtmp", bufs=2)
                    nc.vector.tensor_mul(t1[:], psr[:], sq[:])
                    nc.vector.tensor_mul(t2[:], qt_s[:].bitcast(F32), cq[:])
                    qrt = ph.tile([128, L], F32R, tag="qrt", bufs=2)
                    nc.vector.tensor_add(qrt[:], t1[:], t2[:])

                    # scores -> E tiles
                    etiles = []
                    for mb in range(NB):
                        w = L - 128 * mb
                        et = ph.tile([128, w], BF16, tag=f"esc{mb}", bufs=3,
                                     name=f"esc_h{mb}")
                        etiles.append(et)
                    for mb in range(NB):
                        kb = slice(mb * 128, (mb + 1) * 128)
                        for (qs, qe) in _chunks_for_stripe(mb):
                            s = qe - qs
                            psA = psp.tile([128, 2 * s], F32, tag="scA", bufs=1,
                                           name="psA")
                            psB = psp.tile([128, 2 * s], F32, tag="scB", bufs=1,
                                           name="psB")
                            nc.tensor.matmul(psA[:, 0:s], krt[g][0:64, kb],
                                             qrt[0:64, qs:qe])
                            nc.tensor.matmul(psA[:, s:2 * s], kswap[g][0:64, kb],
                                             qrt[0:64, qs:qe])
                            nc.tensor.matmul(psB[:, 0:s], krt[g][64:128, kb],
                                             qrt[64:128, qs:qe])
                            nc.tensor.matmul(psB[:, s:2 * s], kswap[g][64:128, kb],
                                             qrt[64:128, qs:qe])
                            bs = ph.tile([128, 2 * s], F32, tag="bs", bufs=3)
                            nc.any.tensor_copy(bs[:], psB[:])
                            tp = ph.tile([128, 2 * s], F32, tag="tprod", bufs=3)
                            nc.vector.tensor_mul(tp[:], psA[:], bs[:])
                            raw = ph.tile([128, s], F32, tag="raw", bufs=3)
                            nc.vector.scalar_tensor_tensor(
                                raw[:], tp[:, s:2 * s], consts["cprime"][:, h:h + 1],
                                tp[:, 0:s], op0=AluOp.mult, op1=AluOp.add)
                            esl = etiles[mb][:, qs - 128 * mb: qe - 128 * mb]
                            nc.scalar.activation(esl, raw[:], Act.Exp,
                                                 bias=consts["maskb"][:, mb:mb + 1],
                                                 scale=consts["alpha"][:, h:h + 1])
                            if qs == 128 * mb:
                                # causal triangle on the diagonal 128 cols
                                nc.gpsimd.affine_select(
                                    etiles[mb][:, 0:128], etiles[mb][:, 0:128],
                                    pattern=[[1, 128]], compare_op=AluOp.is_ge,
                                    fill=0.0, base=0, channel_multiplier=-1)

                    # attnv + rowsum
                    ps_o = psp.tile([128, L], F32, tag="acco", bufs=1, name="ps_o")
                    ps_rs = psp.tile([128, L], F32, tag="accr", bufs=1, name="ps_rs")
                    for c in range(2):
                        mbs = [mb for mb in range(NB) if 128 * mb < 512 * (c + 1)]
                        for i, mb in enumerate(mbs):
                            os_ = max(512 * c, 128 * mb)
                            oe = 512 * (c + 1)
                            esl = etiles[mb][:, os_ - 128 * mb: oe - 128 * mb]
                            st, sp = (i == 0), (i == len(mbs) - 1)
                            nc.tensor.matmul(ps_o[:, os_:oe], vblk[g * NB + mb][:],
                                             esl, start=st, stop=sp)
                            nc.tensor.matmul(ps_rs[:, os_:oe], consts["onesb"][:],
                                             esl, start=st, stop=sp)
                    rcp = ph.tile([128, L], F32, tag="rcp", bufs=1)
                    nc.vector.reciprocal_approx_fast(rcp[:], ps_rs[:])
                    nc.vector.tensor_mul(outtn[h][:], ps_o[:], rcp[:])

            # ---------------- epilogue: Wo projection
            with tc.tile_pool(name="ep", bufs=1) as pe:
                for lb in range(NB):
                    for c in range(2):
                        psy = psp.tile([128, 1024], F32, tag="pj", bufs=1, name="psy")
                        for cc in range(2):
                            for hh in range(NH):
                                nc.tensor.matmul(
                                    psy[:, cc * 512:(cc + 1) * 512],
                                    outtn[hh][:, lb * 128:(lb + 1) * 128],
                                    wo_t[hh][:, c * 1024 + cc * 512:
                                            c * 1024 + (cc + 1) * 512],
                                    start=(hh == 0), stop=(hh == NH - 1))
                        yt = pe.tile([128, 1024], BF16, tag="ytile", bufs=3)
                        nc.any.tensor_copy(yt[:], psy[:])
                        nc.sync.dma_start(
                            y[lb * 128:(lb + 1) * 128, c * 1024:(c + 1) * 1024], yt[:])

    nc.compile()
    return nc


def _host_prep(x, Wq, Wk, Wv, Wo, q_param, log_scale, cos, sin, mask):
    """Build the 8 per-core input maps."""
    x = np.asarray(x, np.float32)
    Wq = np.asarray(Wq, np.float32)
    Wk = np.asarray(Wk, np.float32)
    Wv = np.asarray(Wv, np.float32)
    Wo = np.asarray(Wo, np.float32)
    cos = np.asarray(cos, np.float32)[0]      # [L, H, 64]
    sin = np.asarray(sin, np.float32)[0]
    qp = np.asarray(q_param, np.float32).reshape(H)
    ls = np.asarray(log_scale, np.float32).reshape(H)
    mask = np.asarray(mask)

    p64 = np.arange(128) % 64

    PM = np.zeros((128, 128), np.float32)
    for dp in range(128):
        base, r = (dp // 64) * 64, dp % 64
        if r < 32:
            PM[base + r + 32, dp] = -1.0
        else:
            PM[base + r - 32, dp] = 1.0
    SW = np.zeros((128, 128), np.float32)
    for dp in range(128):
        SW[(dp + 64) % 128, dp] = 1.0
    ONES = np.ones((128, 128), ml_dtypes.bfloat16)
    IDENT = np.eye(128, dtype=ml_dtypes.bfloat16)

    in_maps = []
    for core in range(8):
        b, g2 = core // 2, core % 2
        heads = list(range(g2 * NH, (g2 + 1) * NH))
        kvs = list(range(g2 * NKV, (g2 + 1) * NKV))

        xb = x[b]
        xh = xb.astype(ml_dtypes.bfloat16)
        xlo = (xb - xh.astype(np.float32)).astype(ml_dtypes.bfloat16)

        wq_c = Wq[:, g2 * NH * 128:(g2 + 1) * NH * 128]
        wk_c = Wk[:, g2 * NKV * 128:(g2 + 1) * NKV * 128]
        wv_c = Wv[:, g2 * NKV * 128:(g2 + 1) * NKV * 128]
        wo_c = Wo[g2 * NH * 128:(g2 + 1) * NH * 128, :]

        wq_p = wq_c.reshape(16, 128, NH * 128).transpose(1, 0, 2).copy()
        wk_p = wk_c.reshape(16, 128, NKV * 128).transpose(1, 0, 2).copy()
        wv_p = wv_c.reshape(16, 128, NKV * 128).transpose(1, 0, 2).copy()
        wo_p = wo_c.reshape(NH, 128, D).transpose(1, 0, 2).astype(ml_dtypes.bfloat16)

        bf = ml_dtypes.bfloat16
        cosq_p = np.ascontiguousarray(
            cos[:, heads, :][:, :, p64].transpose(2, 1, 0)).astype(bf)
        sinq_p = np.ascontiguousarray(
            sin[:, heads, :][:, :, p64].transpose(2, 1, 0)).astype(bf)
        cosk_p = np.ascontiguousarray(
            cos[:, kvs, :][:, :, p64].transpose(2, 1, 0)).astype(bf)
        sink_p = np.ascontiguousarray(
            sin[:, kvs, :][:, :, p64].transpose(2, 1, 0)).astype(bf)

        mb = np.where(mask[b].reshape(NB, 128).T.astype(bool), 0.0, -1e9)
        mb = mb.astype(np.float32)

        cpr = np.tile((-2.0 * np.tanh(qp[heads]))[None, :], (128, 1))
        alp = np.tile((np.exp(ls[heads]) / HD)[None, :], (128, 1))

        in_maps.append({
            "xh": xh, "xl": xlo,
            "wq": wq_p.astype(np.float32), "wk": wk_p.astype(np.float32),
            "wv": wv_p.astype(np.float32), "wo": wo_p,
            "cosq": cosq_p, "sinq": sinq_p, "cosk": cosk_p, "sink": sink_p,
            "maskb": mb, "cprime": cpr.astype(np.float32),
            "alpha": alp.astype(np.float32),
            "pmrot": PM.astype(ml_dtypes.bfloat16),
            "pmswap": SW.astype(ml_dtypes.bfloat16),
            "onesb": ONES, "identb": IDENT,
        })
    return in_maps


def kernel(**inputs):
    if "nc" not in _CACHED:
        _CACHED["nc"] = build_program()
    nc = _CACHED["nc"]
    in_maps = _host_prep(**inputs)
    res = run_bass_kernel_spmd(nc, in_maps, list(range(8))).results
    out = np.empty((B, L, D), np.float32)
    for b in range(B):
        out[b] = (res[2 * b]["y"].astype(np.float32)
                  + res[2 * b + 1]["y"].astype(np.float32))
    return out


# revision 56
# speedup vs baseline: 1.4107x; 1.0626x over previous
"""BivectorRotarySelfAttention TRN2 kernel.

Sharding: 8 cores = 4 batches x 2 head-halves. Each core computes one batch's
attention for 8 heads (2 kv heads) and a partial output projection; host sums
the two head-half partials per batch.

Per-core dataflow (transposed layouts: features in partitions, seq in free):
  xT    = recombine(dma_transpose(x_hi), dma_transpose(x_lo))      [f32r]
  qT/kT/vT = W-blocks.T @ xT   (PSUM-accumulated f32r matmuls)
  rope via PE permutation-matmul + 2 DVE muls + 1 add
  scores S^T[m,q]: 4 K=64 matmuls (S0,S1 / C0,C1 row-packed pairs)
  raw = S0*S1 + c'*C0*C1 ; E = exp(alpha*raw + key_mask_bias)  [bf16]
  causal: affine_select on diagonal blocks (GPSIMD)
  outT[d,q] = v-blocks.T @ E (bf16), rowsums via ones-matmul broadcast
  y[l,:] += (outT_h * recip_rowsum) @ Wo_h   (bf16)
"""
import sys
if '/opt/trn_rl_repo' not in sys.path:
    sys.path.insert(0, '/opt/trn_rl_repo')

import numpy as np
import ml_dtypes

import concourse.bass as bass
import concourse.mybir as mybir
import concourse.tile as tile
from concourse import bacc
from concourse.bass_utils import run_bass_kernel_spmd

F32 = mybir.dt.float32
F32R = mybir.dt.float32r
BF16 = mybir.dt.bfloat16

B, L, D, H, HKV = 4, 1024, 2048, 16, 4
HD = D // H            # 128
HD2 = HD // 2          # 64
NH = 8                 # heads per core
NKV = 2                # kv heads per core
NB = L // 128          # 8 blocks of 128
AluOp = mybir.AluOpType
Act = mybir.ActivationFunctionType

_CACHED = {}


def _chunks_for_stripe(mb):
    """Q-column chunks [(qs, qe)] covering [128*mb, 1024), split at 256-multiples."""
    q0 = 128 * mb
    out = []
    while q0 < L:
        qe = min(L, (q0 // 256 + 1) * 256)
        out.append((q0, qe))
        q0 = qe
    return out


def build_program():
    nc = bacc.Bacc("TRN2", target_bir_lowering=False, debug=False)

    # ---- dram params (per-core shapes) ----
    xh = nc.declare_dram_parameter("xh", [L, D], BF16, isOutput=False)
    xl = nc.declare_dram_parameter("xl", [L, D], BF16, isOutput=False)
    wq = nc.declare_dram_parameter("wq", [128, 16, NH * 128], F32R, isOutput=False)
    wk = nc.declare_dram_parameter("wk", [128, 16, NKV * 128], F32R, isOutput=False)
    wv = nc.declare_dram_parameter("wv", [128, 16, NKV * 128], F32R, isOutput=False)
    wo = nc.declare_dram_parameter("wo", [128, NH, D], BF16, isOutput=False)
    cosq = nc.declare_dram_parameter("cosq", [128, NH, L], BF16, isOutput=False)
    sinq = nc.declare_dram_parameter("sinq", [128, NH, L], BF16, isOutput=False)
    cosk = nc.declare_dram_parameter("cosk", [128, NKV, L], BF16, isOutput=False)
    sink = nc.declare_dram_parameter("sink", [128, NKV, L], BF16, isOutput=False)
    maskb = nc.declare_dram_parameter("maskb", [128, NB], F32, isOutput=False)
    cprime = nc.declare_dram_parameter("cprime", [128, NH], F32, isOutput=False)
    alpha = nc.declare_dram_parameter("alpha", [128, NH], F32, isOutput=False)
    pmrot = nc.declare_dram_parameter("pmrot", [128, 128], BF16, isOutput=False)
    pmswap = nc.declare_dram_parameter("pmswap", [128, 128], BF16, isOutput=False)
    onesb = nc.declare_dram_parameter("onesb", [128, 128], BF16, isOutput=False)
    identb = nc.declare_dram_parameter("identb", [128, 128], BF16, isOutput=False)
    y = nc.declare_dram_parameter("y", [L, D], BF16, isOutput=True)

    with tile.TileContext(nc) as tc:
        with (
            tc.tile_pool(name="persist", bufs=1) as pp,
            tc.tile_pool(name="psum", bufs=1, space="PSUM") as psp,
        ):
            # persistent tiles
            consts = {}
            for nm, src, dt_ in [("pmrot", pmrot, BF16), ("pmswap", pmswap, BF16),
                                 ("onesb", onesb, BF16), ("identb", identb, BF16),
                                 ("maskb", maskb, F32), ("cprime", cprime, F32),
                                 ("alpha", alpha, F32)]:
                t = pp.tile(list(src.shape), dt_, tag=nm, name=nm)
                nc.sync.dma_start(t[:], src[:])
                consts[nm] = t

            xt = [pp.tile([128, L], F32R, tag=f"xt{ib}", name=f"xt{ib}")
                  for ib in range(16)]
            krt = [pp.tile([128, L], BF16, tag=f"krt{g}", name=f"krt{g}")
                   for g in range(NKV)]
            kswap = [pp.tile([128, L], BF16, tag=f"ksw{g}", name=f"ksw{g}")
                     for g in range(NKV)]
            vblk = [pp.tile([128, 128], BF16, tag=f"vb{i}", name=f"vb{i}")
                    for i in range(NKV * NB)]
            outtn = [pp.tile([128, L], BF16, tag=f"ot{h}", name=f"ot{h}")
                     for h in range(NH)]
            wo_t = [pp.tile([128, D], BF16, tag=f"wo{h}", name=f"wo{h}")
                    for h in range(NH)]

            # ---------------- prologue: xT + k/v proj + k rope + v transpose
            with tc.tile_pool(name="pro", bufs=1) as ppro:
                # x transpose-load + recombine
                for ib in range(16):
                    th = ppro.tile([128, L], BF16, tag="xh_t", bufs=3)
                    tl = ppro.tile([128, L], BF16, tag="xl_t", bufs=3)
                    nc.sync.dma_start_transpose(th[:], xh[:, ib * 128:(ib + 1) * 128])
                    nc.sync.dma_start_transpose(tl[:], xl[:, ib * 128:(ib + 1) * 128])
                    nc.vector.tensor_add(xt[ib][:], th[:], tl[:])

                wk_t = ppro.tile([128, 16, NKV * 128], F32R, tag="wk")
                wv_t = ppro.tile([128, 16, NKV * 128], F32R, tag="wv")
                nc.scalar.dma_start(wk_t[:], wk[:])
                nc.scalar.dma_start(wv_t[:], wv[:])

                kt_s = []
                for g in range(NKV):
                    ps = psp.tile([128, L], F32, tag="pj", bufs=1)
                    for ib in range(16):
                        for c in range(2):
                            nc.tensor.matmul(
                                ps[:, c * 512:(c + 1) * 512],
                                wk_t[:, ib, g * 128:(g + 1) * 128],
                                xt[ib][:, c * 512:(c + 1) * 512],
                                start=(ib == 0), stop=(ib == 15))
                    kt = ppro.tile([128, L], BF16, tag="kt_s", bufs=2)
                    nc.any.tensor_copy(kt[:], ps[:])
                    kt_s.append(kt)

                # k rope
                for g in range(NKV):
                    psr = psp.tile([128, L], F32, tag="pj", bufs=1)
                    for c in range(2):
                        nc.tensor.matmul(psr[:, c * 512:(c + 1) * 512],
                                         consts["pmrot"][:],
                                         kt_s[g][:, c * 512:(c + 1) * 512])
                    t1 = ppro.tile([128, L], BF16, tag="rtmp", bufs=4)
                    t2 = ppro.tile([128, L], BF16, tag="rtmp", bufs=4)
                    csl = ppro.tile([128, L], BF16, tag="ktab", bufs=4)
                    snl = ppro.tile([128, L], BF16, tag="ktab", bufs=4)
                    nc.sync.dma_start(csl[:], cosk[:, g, :])
                    nc.sync.dma_start(snl[:], sink[:, g, :])
                    nc.vector.tensor_mul(t1[:], psr[:], snl[:])
                    nc.vector.tensor_mul(t2[:], kt_s[g][:], csl[:])
                    nc.vector.tensor_add(krt[g][:], t1[:], t2[:])
                    # kswap = partition-swap of krt
                    psw = psp.tile([128, L], F32, tag="pj", bufs=1)
                    for c in range(2):
                        nc.tensor.matmul(psw[:, c * 512:(c + 1) * 512],
                                         consts["pmswap"][:],
                                         krt[g][:, c * 512:(c + 1) * 512])
                    nc.any.tensor_copy(kswap[g][:], psw[:])

                # v proj (bf16 out) + transpose to [m, d] blocks
                for g in range(NKV):
                    ps = psp.tile([128, L], F32, tag="pj", bufs=1)
                    for ib in range(16):
                        for c in range(2):
                            nc.tensor.matmul(
                                ps[:, c * 512:(c + 1) * 512],
                                wv_t[:, ib, g * 128:(g + 1) * 128],
                                xt[ib][:, c * 512:(c + 1) * 512],
                                start=(ib == 0), stop=(ib == 15))
                    vt = ppro.tile([128, L], BF16, tag="vt_s", bufs=2)
                    nc.any.tensor_copy(vt[:], ps[:])
                    for mb in range(NB):
                        pv = psp.tile([128, 128], BF16, tag="pj", bufs=1)
                        nc.tensor.transpose(pv[:], vt[:, mb * 128:(mb + 1) * 128],
                                            consts["identb"][:])
                        nc.vector.tensor_copy(vblk[g * NB + mb][:], pv[:])

            # ---------------- head loop
            with tc.tile_pool(name="hl", bufs=1) as ph:
                for h in range(NH):
                    g = h // 4  # local kv head
                    wq_t = ph.tile([128, 16, 128], F32R, tag="wq_h", bufs=2)
                    nc.sync.dma_start(wq_t[:], wq[:, :, h * 128:(h + 1) * 128])
                    cq = ph.tile([128, L], F32, tag="tabq", bufs=2)
                    sq = ph.tile([128, L], F32, tag="tabq", bufs=2)
                    nc.sync.dma_start(cq[:], cosq[:, h, :])
                    nc.sync.dma_start(sq[:], sinq[:, h, :])

                    psq = psp.tile([128, L], F32, tag="pj", bufs=1)
                    for ib in range(16):
                        for c in range(2):
                            nc.tensor.matmul(
                                psq[:, c * 512:(c + 1) * 512],
                                wq_t[:, ib, :],
                                xt[ib][:, c * 512:(c + 1) * 512],
                                start=(ib == 0), stop=(ib == 15))
                    qt_s = ph.tile([128, L], F32R, tag="qt_s", bufs=2)
                    nc.any.tensor_copy(qt_s[:], psq[:])

                    psr = psp.tile([128, L], F32, tag="pj", bufs=1)
                    for c in range(2):
                        nc.tensor.matmul(psr[:, c * 512:(c + 1) * 512],
                                         consts["pmrot"][:],
                                         qt_s[:, c * 512:(c + 1) * 512])
                    t1 = ph.tile([128, L], F32, tag="qtmp", bufs=2)
                    t2 = ph.tile([128, L], F32, tag="qtmp", bufs=2)
                    nc.vector.tensor_mul(t1[:], psr[:], sq[:])
                    nc.vector.tensor_mul(t2[:], qt_s[:].bitcast(F32), cq[:])
                    qrt = ph.tile([128, L], F32R, tag="qrt", bufs=2)
                    nc.vector.tensor_add(qrt[:], t1[:], t2[:])

                    # scores -> E tiles
                    etiles = []
                    for mb in range(NB):
                        w = L - 128 * mb
                        et = ph.tile([128, w], BF16, tag=f"esc{mb}", bufs=3,
                                     name=f"esc_h{mb}")
                        etiles.append(et)
                    for mb in range(NB):
                        kb = slice(mb * 128, (mb + 1) * 128)
                        for (qs, qe) in _chunks_for_stripe(mb):
                            s = qe - qs
                            psA = psp.tile([128, 2 * s], F32, tag="scA", bufs=1,
                                           name="psA")
                            psB = psp.tile([128, 2 * s], F32, tag="scB", bufs=1,
                                           name="psB")
                            nc.tensor.matmul(psA[:, 0:s], krt[g][0:64, kb],
                                             qrt[0:64, qs:qe])
                            nc.tensor.matmul(psA[:, s:2 * s], kswap[g][0:64, kb],
                                             qrt[0:64, qs:qe])
                            nc.tensor.matmul(psB[:, 0:s], krt[g][64:128, kb],
                                             qrt[64:128, qs:qe])
                            nc.tensor.matmul(psB[:, s:2 * s], kswap[g][64:128, kb],
                                             qrt[64:128, qs:qe])
                            bs = ph.tile([128, 2 * s], F32, tag="bs", bufs=3)
                            nc.any.tensor_copy(bs[:], psB[:])
                            tp = ph.tile([128, 2 * s], F32, tag="tprod", bufs=3)
                            nc.vector.tensor_mul(tp[:], psA[:], bs[:])
                            raw = ph.tile([128, s], F32, tag="raw", bufs=3)
                            nc.vector.scalar_tensor_tensor(
                                raw[:], tp[:, s:2 * s], consts["cprime"][:, h:h + 1],
                                tp[:, 0:s], op0=AluOp.mult, op1=AluOp.add)
                            esl = etiles[mb][:, qs - 128 * mb: qe - 128 * mb]
                            nc.scalar.activation(esl, raw[:], Act.Exp,
                                                 bias=consts["maskb"][:, mb:mb + 1],
                                                 scale=consts["alpha"][:, h:h + 1])
                            if qs == 128 * mb:
                                # causal triangle on the diagonal 128 cols
                                nc.gpsimd.affine_select(
                                    etiles[mb][:, 0:128], etiles[mb][:, 0:128],
                                    pattern=[[1, 128]], compare_op=AluOp.is_ge,
                                    fill=0.0, base=0, channel_multiplier=-1)

                    # attnv + rowsum
                    ps_o = psp.tile([128, L], F32, tag="acco", bufs=1, name="ps_o")
                    ps_rs = psp.tile([128, L], F32, tag="accr", bufs=1, name="ps_rs")
                    for c in range(2):
                        mbs = [mb for mb in range(NB) if 128 * mb < 512 * (c + 1)]
                        for i, mb in enumerate(mbs):
                            os_ = max(512 * c, 128 * mb)
                            oe = 512 * (c + 1)
                            esl = etiles[mb][:, os_ - 128 * mb: oe - 128 * mb]
                            st, sp = (i == 0), (i == len(mbs) - 1)
                            nc.tensor.matmul(ps_o[:, os_:oe], vblk[g * NB + mb][:],
                                             esl, start=st, stop=sp)
                            nc.tensor.matmul(ps_rs[:, os_:oe], consts["onesb"][:],
                                             esl, start=st, stop=sp)
                    rcp = ph.tile([128, L], F32, tag="rcp", bufs=1)
                    nc.vector.reciprocal_approx_fast(rcp[:], ps_rs[:])
                    nc.vector.tensor_mul(outtn[h][:], ps_o[:], rcp[:])

            # ---------------- epilogue: Wo projection
            with tc.tile_pool(name="ep", bufs=1) as pe:
                for lb in range(NB):
                    for c in range(2):
                        # reuse the double-buffered score-chunk psum tags so
                        # consecutive output groups overlap with the yt copies
                        psyA = psp.tile([128, 512], F32, tag="scA", bufs=2,
                                        name="psyA")
                        psyB = psp.tile([128, 512], F32, tag="scB", bufs=2,
                                        name="psyB")
                        for cc, pt in ((0, psyA), (1, psyB)):
                            for hh in range(NH):
                                nc.tensor.matmul(
                                    pt[:],
                                    outtn[hh][:, lb * 128:(lb + 1) * 128],
                                    wo_t[hh][:, c * 1024 + cc * 512:
                                            c * 1024 + (cc + 1) * 512],
                                    start=(hh == 0), stop=(hh == NH - 1))
                        yt = pe.tile([128, 1024], BF16, tag="ytile", bufs=3)
                        nc.any.tensor_copy(yt[:, 0:512], psyA[:])
                        nc.any.tensor_copy(yt[:, 512:1024], psyB[:])
                        nc.sync.dma_start(
                            y[lb * 128:(lb + 1) * 128, c * 1024:(c + 1) * 1024], yt[:])

    nc.compile()
    return nc


def _host_prep(x, Wq, Wk, Wv, Wo, q_param, log_scale, cos, sin, mask):
    """Build the 8 per-core input maps."""
    x = np.asarray(x, np.float32)
    Wq = np.asarray(Wq, np.float32)
    Wk = np.asarray(Wk, np.float32)
    Wv = np.asarray(Wv, np.float32)
    Wo = np.asarray(Wo, np.float32)
    cos = np.asarray(cos, np.float32)[0]      # [L, H, 64]
    sin = np.asarray(sin, np.float32)[0]
    qp = np.asarray(q_param, np.float32).reshape(H)
    ls = np.asarray(log_scale, np.float32).reshape(H)
    mask = np.asarray(mask)

    p64 = np.arange(128) % 64

    PM = np.zeros((128, 128), np.float32)
    for dp in range(128):
        base, r = (dp // 64) * 64, dp % 64
        if r < 32:
            PM[base + r + 32, dp] = -1.0
        else:
            PM[base + r - 32, dp] = 1.0
    SW = np.zeros((128, 128), np.float32)
    for dp in range(128):
        SW[(dp + 64) % 128, dp] = 1.0
    ONES = np.ones((128, 128), ml_dtypes.bfloat16)
    IDENT = np.eye(128, dtype=ml_dtypes.bfloat16)

    in_maps = []
    for core in range(8):
        b, g2 = core // 2, core % 2
        heads = list(range(g2 * NH, (g2 + 1) * NH))
        kvs = list(range(g2 * NKV, (g2 + 1) * NKV))

        xb = x[b]
        xh = xb.astype(ml_dtypes.bfloat16)
        xlo = (xb - xh.astype(np.float32)).astype(ml_dtypes.bfloat16)

        wq_c = Wq[:, g2 * NH * 128:(g2 + 1) * NH * 128]
        wk_c = Wk[:, g2 * NKV * 128:(g2 + 1) * NKV * 128]
        wv_c = Wv[:, g2 * NKV * 128:(g2 + 1) * NKV * 128]
        wo_c = Wo[g2 * NH * 128:(g2 + 1) * NH * 128, :]

        wq_p = wq_c.reshape(16, 128, NH * 128).transpose(1, 0, 2).copy()
        wk_p = wk_c.reshape(16, 128, NKV * 128).transpose(1, 0, 2).copy()
        wv_p = wv_c.reshape(16, 128, NKV * 128).transpose(1, 0, 2).copy()
        wo_p = wo_c.reshape(NH, 128, D).transpose(1, 0, 2).astype(ml_dtypes.bfloat16)

        bf = ml_dtypes.bfloat16
        cosq_p = np.ascontiguousarray(
            cos[:, heads, :][:, :, p64].transpose(2, 1, 0)).astype(bf)
        sinq_p = np.ascontiguousarray(
            sin[:, heads, :][:, :, p64].transpose(2, 1, 0)).astype(bf)
        cosk_p = np.ascontiguousarray(
            cos[:, kvs, :][:, :, p64].transpose(2, 1, 0)).astype(bf)
        sink_p = np.ascontiguousarray(
            sin[:, kvs, :][:, :, p64].transpose(2, 1, 0)).astype(bf)

        mb = np.where(mask[b].reshape(NB, 128).T.astype(bool), 0.0, -1e9)
        mb = mb.astype(np.float32)

        cpr = np.tile((-2.0 * np.tanh(qp[heads]))[None, :], (128, 1))
        alp = np.tile((np.exp(ls[heads]) / HD)[None, :], (128, 1))

        in_maps.append({
            "xh": xh, "xl": xlo,
            "wq": wq_p.astype(np.float32), "wk": wk_p.astype(np.float32),
            "wv": wv_p.astype(np.float32), "wo": wo_p,
            "cosq": cosq_p, "sinq": sinq_p, "cosk": cosk_p, "sink": sink_p,
            "maskb": mb, "cprime": cpr.astype(np.float32),
            "alpha": alp.astype(np.float32),
            "pmrot": PM.astype(ml_dtypes.bfloat16),
            "pmswap": SW.astype(ml_dtypes.bfloat16),
            "onesb": ONES, "identb": IDENT,
        })
    return in_maps


def kernel(**inputs):
    if "nc" not in _CACHED:
        _CACHED["nc"] = build_program()
    nc = _CACHED["nc"]
    in_maps = _host_prep(**inputs)
    res = run_bass_kernel_spmd(nc, in_maps, list(range(8))).results
    out = np.empty((B, L, D), np.float32)
    for b in range(B):
        out[b] = (res[2 * b]["y"].astype(np.float32)
                  + res[2 * b + 1]["y"].astype(np.float32))
    return out


# revision 59
# speedup vs baseline: 1.5052x; 1.0670x over previous
"""BivectorRotarySelfAttention TRN2 kernel.

Sharding: 8 cores = 4 batches x 2 head-halves. Each core computes one batch's
attention for 8 heads (2 kv heads) and a partial output projection; host sums
the two head-half partials per batch.

Per-core dataflow (transposed layouts: features in partitions, seq in free):
  xT    = recombine(dma_transpose(x_hi), dma_transpose(x_lo))      [f32r]
  qT/kT/vT = W-blocks.T @ xT   (PSUM-accumulated f32r matmuls)
  rope via PE permutation-matmul + 2 DVE muls + 1 add
  scores S^T[m,q]: 4 K=64 matmuls (S0,S1 / C0,C1 row-packed pairs)
  raw = S0*S1 + c'*C0*C1 ; E = exp(alpha*raw + key_mask_bias)  [bf16]
  causal: affine_select on diagonal blocks (GPSIMD)
  outT[d,q] = v-blocks.T @ E (bf16), rowsums via ones-matmul broadcast
  y[l,:] += (outT_h * recip_rowsum) @ Wo_h   (bf16)
"""
import sys
if '/opt/trn_rl_repo' not in sys.path:
    sys.path.insert(0, '/opt/trn_rl_repo')

import numpy as np
import ml_dtypes

import concourse.bass as bass
import concourse.mybir as mybir
import concourse.tile as tile
from concourse import bacc
from concourse.bass_utils import run_bass_kernel_spmd

F32 = mybir.dt.float32
F32R = mybir.dt.float32r
BF16 = mybir.dt.bfloat16

B, L, D, H, HKV = 4, 1024, 2048, 16, 4
HD = D // H            # 128
HD2 = HD // 2          # 64
NH = 8                 # heads per core
NKV = 2                # kv heads per core
NB = L // 128          # 8 blocks of 128
AluOp = mybir.AluOpType
Act = mybir.ActivationFunctionType

_CACHED = {}


def _chunks_for_stripe(mb):
    """Q-column chunks [(qs, qe)] covering [128*mb, 1024), split at 256-multiples."""
    q0 = 128 * mb
    out = []
    while q0 < L:
        qe = min(L, (q0 // 256 + 1) * 256)
        out.append((q0, qe))
        q0 = qe
    return out


def build_program():
    nc = bacc.Bacc("TRN2", target_bir_lowering=False, debug=False)

    # ---- dram params (per-core shapes) ----
    xh = nc.declare_dram_parameter("xh", [L, D], BF16, isOutput=False)
    xl = nc.declare_dram_parameter("xl", [L, D], BF16, isOutput=False)
    wq = nc.declare_dram_parameter("wq", [128, 16, NH * 128], F32R, isOutput=False)
    wk = nc.declare_dram_parameter("wk", [128, 16, NKV * 128], F32R, isOutput=False)
    wv = nc.declare_dram_parameter("wv", [128, 16, NKV * 128], F32R, isOutput=False)
    wo = nc.declare_dram_parameter("wo", [128, NH, D], BF16, isOutput=False)
    cosq = nc.declare_dram_parameter("cosq", [128, NH, L], BF16, isOutput=False)
    sinq = nc.declare_dram_parameter("sinq", [128, NH, L], BF16, isOutput=False)
    cosk = nc.declare_dram_parameter("cosk", [128, NKV, L], BF16, isOutput=False)
    sink = nc.declare_dram_parameter("sink", [128, NKV, L], BF16, isOutput=False)
    maskb = nc.declare_dram_parameter("maskb", [128, NB], F32, isOutput=False)
    cprime = nc.declare_dram_parameter("cprime", [128, NH], F32, isOutput=False)
    alpha = nc.declare_dram_parameter("alpha", [128, NH], F32, isOutput=False)
    pmrot = nc.declare_dram_parameter("pmrot", [128, 128], BF16, isOutput=False)
    pmswap = nc.declare_dram_parameter("pmswap", [128, 128], BF16, isOutput=False)
    onesb = nc.declare_dram_parameter("onesb", [128, 128], BF16, isOutput=False)
    identb = nc.declare_dram_parameter("identb", [128, 128], BF16, isOutput=False)
    y = nc.declare_dram_parameter("y", [L, D], BF16, isOutput=True)

    with tile.TileContext(nc) as tc:
        with (
            tc.tile_pool(name="persist", bufs=1) as pp,
            tc.tile_pool(name="psum", bufs=1, space="PSUM") as psp,
        ):
            # persistent tiles
            consts = {}
            for nm, src, dt_ in [("pmrot", pmrot, BF16), ("pmswap", pmswap, BF16),
                                 ("onesb", onesb, BF16), ("identb", identb, BF16),
                                 ("maskb", maskb, F32), ("cprime", cprime, F32),
                                 ("alpha", alpha, F32)]:
                t = pp.tile(list(src.shape), dt_, tag=nm, name=nm)
                nc.sync.dma_start(t[:], src[:])
                consts[nm] = t

            xt = [pp.tile([128, L], F32R, tag=f"xt{ib}", name=f"xt{ib}")
                  for ib in range(16)]
            krt = [pp.tile([128, L], BF16, tag=f"krt{g}", name=f"krt{g}")
                   for g in range(NKV)]
            kswap = [pp.tile([128, L], BF16, tag=f"ksw{g}", name=f"ksw{g}")
                     for g in range(NKV)]
            vblk = [pp.tile([128, 128], BF16, tag=f"vb{i}", name=f"vb{i}")
                    for i in range(NKV * NB)]
            outtn = [pp.tile([128, L], BF16, tag=f"ot{h}", name=f"ot{h}")
                     for h in range(NH)]
            wo_t = [pp.tile([128, D], BF16, tag=f"wo{h}", name=f"wo{h}")
                    for h in range(NH)]

            # ---------------- prologue: xT + k/v proj + k rope + v transpose
            with tc.tile_pool(name="pro", bufs=1) as ppro:
                # x transpose-load + recombine
                for ib in range(16):
                    th = ppro.tile([128, L], BF16, tag="xh_t", bufs=3)
                    tl = ppro.tile([128, L], BF16, tag="xl_t", bufs=3)
                    nc.sync.dma_start_transpose(th[:], xh[:, ib * 128:(ib + 1) * 128])
                    nc.sync.dma_start_transpose(tl[:], xl[:, ib * 128:(ib + 1) * 128])
                    nc.vector.tensor_add(xt[ib][:], th[:], tl[:])

                wk_t = ppro.tile([128, 16, NKV * 128], F32R, tag="wk")
                wv_t = ppro.tile([128, 16, NKV * 128], F32R, tag="wv")
                nc.scalar.dma_start(wk_t[:], wk[:])
                nc.scalar.dma_start(wv_t[:], wv[:])

                kt_s = []
                # k (both kv heads) and v g=0 projections accumulate in
                # parallel streams paced by the xt arrivals; v g=1 follows,
                # overlapped with the k-rope chains. PSUM comes from the
                # head-loop tags (pj / scA+scB / acco+accr halves).
                ps_k0 = psp.tile([128, L], F32, tag="pj", bufs=1, name="ps_k0")
                ps_k1 = [psp.tile([128, 512], F32, tag=t, bufs=2, name=f"ps_k1{c}")
                         for c, t in ((0, "scA"), (1, "scB"))]
                ps_v0 = [psp.tile([128, 512], F32, tag=t, bufs=1, name=f"ps_v0{c}")
                         for c, t in ((0, "acco"), (1, "accr"))]
                for ib in range(16):
                    for c in range(2):
                        nc.tensor.matmul(
                            ps_k0[:, c * 512:(c + 1) * 512],
                            wk_t[:, ib, 0:128],
                            xt[ib][:, c * 512:(c + 1) * 512],
                            start=(ib == 0), stop=(ib == 15))
                    for c in range(2):
                        nc.tensor.matmul(
                            ps_k1[c][:],
                            wk_t[:, ib, 128:256],
                            xt[ib][:, c * 512:(c + 1) * 512],
                            start=(ib == 0), stop=(ib == 15))
                    for c in range(2):
                        nc.tensor.matmul(
                            ps_v0[c][:],
                            wv_t[:, ib, 0:128],
                            xt[ib][:, c * 512:(c + 1) * 512],
                            start=(ib == 0), stop=(ib == 15))
                kt0 = ppro.tile([128, L], BF16, tag="kt_s", bufs=2)
                nc.any.tensor_copy(kt0[:], ps_k0[:])
                kt1 = ppro.tile([128, L], BF16, tag="kt_s", bufs=2)
                nc.any.tensor_copy(kt1[:, 0:512], ps_k1[0][:])
                nc.any.tensor_copy(kt1[:, 512:1024], ps_k1[1][:])
                kt_s = [kt0, kt1]
                vt0 = ppro.tile([128, L], BF16, tag="vt_s", bufs=2)
                nc.any.tensor_copy(vt0[:, 0:512], ps_v0[0][:])
                nc.any.tensor_copy(vt0[:, 512:1024], ps_v0[1][:])

                # k rope
                for g in range(NKV):
                    psr = psp.tile([128, L], F32, tag="pj", bufs=1)
                    for c in range(2):
                        nc.tensor.matmul(psr[:, c * 512:(c + 1) * 512],
                                         consts["pmrot"][:],
                                         kt_s[g][:, c * 512:(c + 1) * 512])
                    t1 = ppro.tile([128, L], BF16, tag="rtmp", bufs=4)
                    t2 = ppro.tile([128, L], BF16, tag="rtmp", bufs=4)
                    csl = ppro.tile([128, L], BF16, tag="ktab", bufs=4)
                    snl = ppro.tile([128, L], BF16, tag="ktab", bufs=4)
                    nc.sync.dma_start(csl[:], cosk[:, g, :])
                    nc.sync.dma_start(snl[:], sink[:, g, :])
                    nc.vector.tensor_mul(t1[:], psr[:], snl[:])
                    nc.vector.tensor_mul(t2[:], kt_s[g][:], csl[:])
                    nc.vector.tensor_add(krt[g][:], t1[:], t2[:])
                    # kswap = partition-swap of krt
                    psw = psp.tile([128, L], F32, tag="pj", bufs=1)
                    for c in range(2):
                        nc.tensor.matmul(psw[:, c * 512:(c + 1) * 512],
                                         consts["pmswap"][:],
                                         krt[g][:, c * 512:(c + 1) * 512])
                    nc.any.tensor_copy(kswap[g][:], psw[:])

                # v g=1 projection (after k streams close), then transposes
                ps_v1 = [psp.tile([128, 512], F32, tag=t, bufs=1, name=f"ps_v1{c}")
                         for c, t in ((0, "acco"), (1, "accr"))]
                for ib in range(16):
                    for c in range(2):
                        nc.tensor.matmul(
                            ps_v1[c][:],
                            wv_t[:, ib, 128:256],
                            xt[ib][:, c * 512:(c + 1) * 512],
                            start=(ib == 0), stop=(ib == 15))
                vt1 = ppro.tile([128, L], BF16, tag="vt_s", bufs=2)
                nc.any.tensor_copy(vt1[:, 0:512], ps_v1[0][:])
                nc.any.tensor_copy(vt1[:, 512:1024], ps_v1[1][:])
                for g, vt in ((0, vt0), (1, vt1)):
                    for mb in range(NB):
                        pv = psp.tile([128, 128], BF16, tag="pj", bufs=1)
                        nc.tensor.transpose(pv[:], vt[:, mb * 128:(mb + 1) * 128],
                                            consts["identb"][:])
                        nc.vector.tensor_copy(vblk[g * NB + mb][:], pv[:])

            # ---------------- head loop
            with tc.tile_pool(name="hl", bufs=1) as ph:
                for h in range(NH):
                    g = h // 4  # local kv head
                    wq_t = ph.tile([128, 16, 128], F32R, tag="wq_h", bufs=2)
                    nc.sync.dma_start(wq_t[:], wq[:, :, h * 128:(h + 1) * 128])
                    cq = ph.tile([128, L], F32, tag="tabq", bufs=2)
                    sq = ph.tile([128, L], F32, tag="tabq", bufs=2)
                    nc.sync.dma_start(cq[:], cosq[:, h, :])
                    nc.sync.dma_start(sq[:], sinq[:, h, :])

                    psq = psp.tile([128, L], F32, tag="pj", bufs=1)
                    for ib in range(16):
                        for c in range(2):
                            nc.tensor.matmul(
                                psq[:, c * 512:(c + 1) * 512],
                                wq_t[:, ib, :],
                                xt[ib][:, c * 512:(c + 1) * 512],
                                start=(ib == 0), stop=(ib == 15))
                    qt_s = ph.tile([128, L], F32R, tag="qt_s", bufs=2)
                    nc.any.tensor_copy(qt_s[:], psq[:])

                    psr = psp.tile([128, L], F32, tag="pj", bufs=1)
                    for c in range(2):
                        nc.tensor.matmul(psr[:, c * 512:(c + 1) * 512],
                                         consts["pmrot"][:],
                                         qt_s[:, c * 512:(c + 1) * 512])
                    t1 = ph.tile([128, L], F32, tag="qtmp", bufs=2)
                    t2 = ph.tile([128, L], F32, tag="qtmp", bufs=2)
                    nc.vector.tensor_mul(t1[:], psr[:], sq[:])
                    nc.vector.tensor_mul(t2[:], qt_s[:].bitcast(F32), cq[:])
                    qrt = ph.tile([128, L], F32R, tag="qrt", bufs=2)
                    nc.vector.tensor_add(qrt[:], t1[:], t2[:])

                    # scores -> E tiles
                    etiles = []
                    for mb in range(NB):
                        w = L - 128 * mb
                        et = ph.tile([128, w], BF16, tag=f"esc{mb}", bufs=3,
                                     name=f"esc_h{mb}")
                        etiles.append(et)
                    for mb in range(NB):
                        kb = slice(mb * 128, (mb + 1) * 128)
                        for (qs, qe) in _chunks_for_stripe(mb):
                            s = qe - qs
                            psA = psp.tile([128, 2 * s], F32, tag="scA", bufs=1,
                                           name="psA")
                            psB = psp.tile([128, 2 * s], F32, tag="scB", bufs=1,
                                           name="psB")
                            nc.tensor.matmul(psA[:, 0:s], krt[g][0:64, kb],
                                             qrt[0:64, qs:qe])
                            nc.tensor.matmul(psA[:, s:2 * s], kswap[g][0:64, kb],
                                             qrt[0:64, qs:qe])
                            nc.tensor.matmul(psB[:, 0:s], krt[g][64:128, kb],
                                             qrt[64:128, qs:qe])
                            nc.tensor.matmul(psB[:, s:2 * s], kswap[g][64:128, kb],
                                             qrt[64:128, qs:qe])
                            bs = ph.tile([128, 2 * s], F32, tag="bs", bufs=3)
                            nc.any.tensor_copy(bs[:], psB[:])
                            tp = ph.tile([128, 2 * s], F32, tag="tprod", bufs=3)
                            nc.vector.tensor_mul(tp[:], psA[:], bs[:])
                            raw = ph.tile([128, s], F32, tag="raw", bufs=3)
                            nc.vector.scalar_tensor_tensor(
                                raw[:], tp[:, s:2 * s], consts["cprime"][:, h:h + 1],
                                tp[:, 0:s], op0=AluOp.mult, op1=AluOp.add)
                            esl = etiles[mb][:, qs - 128 * mb: qe - 128 * mb]
                            nc.scalar.activation(esl, raw[:], Act.Exp,
                                                 bias=consts["maskb"][:, mb:mb + 1],
                                                 scale=consts["alpha"][:, h:h + 1])
                            if qs == 128 * mb:
                                # causal triangle on the diagonal 128 cols
                                nc.gpsimd.affine_select(
                                    etiles[mb][:, 0:128], etiles[mb][:, 0:128],
                                    pattern=[[1, 128]], compare_op=AluOp.is_ge,
                                    fill=0.0, base=0, channel_multiplier=-1)

                    # attnv + rowsum
                    ps_o = psp.tile([128, L], F32, tag="acco", bufs=1, name="ps_o")
                    ps_rs = psp.tile([128, L], F32, tag="accr", bufs=1, name="ps_rs")
                    for c in range(2):
                        mbs = [mb for mb in range(NB) if 128 * mb < 512 * (c + 1)]
                        for i, mb in enumerate(mbs):
                            os_ = max(512 * c, 128 * mb)
                            oe = 512 * (c + 1)
                            esl = etiles[mb][:, os_ - 128 * mb: oe - 128 * mb]
                            st, sp = (i == 0), (i == len(mbs) - 1)
                            nc.tensor.matmul(ps_o[:, os_:oe], vblk[g * NB + mb][:],
                                             esl, start=st, stop=sp)
                            nc.tensor.matmul(ps_rs[:, os_:oe], consts["onesb"][:],
                                             esl, start=st, stop=sp)
                    rcp = ph.tile([128, L], F32, tag="rcp", bufs=1)
                    nc.vector.reciprocal_approx_fast(rcp[:], ps_rs[:])
                    nc.vector.tensor_mul(outtn[h][:], ps_o[:], rcp[:])

            # ---------------- epilogue: Wo projection
            with tc.tile_pool(name="ep", bufs=1) as pe:
                for lb in range(NB):
                    for c in range(2):
                        # reuse the double-buffered score-chunk psum tags so
                        # consecutive output groups overlap with the yt copies
                        psyA = psp.tile([128, 512], F32, tag="scA", bufs=2,
                                        name="psyA")
                        psyB = psp.tile([128, 512], F32, tag="scB", bufs=2,
                                        name="psyB")
                        for cc, pt in ((0, psyA), (1, psyB)):
                            for hh in range(NH):
                                nc.tensor.matmul(
                                    pt[:],
                                    outtn[hh][:, lb * 128:(lb + 1) * 128],
                                    wo_t[hh][:, c * 1024 + cc * 512:
                                            c * 1024 + (cc + 1) * 512],
                                    start=(hh == 0), stop=(hh == NH - 1))
                        yt = pe.tile([128, 1024], BF16, tag="ytile", bufs=3)
                        nc.any.tensor_copy(yt[:, 0:512], psyA[:])
                        nc.any.tensor_copy(yt[:, 512:1024], psyB[:])
                        nc.sync.dma_start(
                            y[lb * 128:(lb + 1) * 128, c * 1024:(c + 1) * 1024], yt[:])

    nc.compile()
    return nc


def _host_prep(x, Wq, Wk, Wv, Wo, q_param, log_scale, cos, sin, mask):
    """Build the 8 per-core input maps."""
    x = np.asarray(x, np.float32)
    Wq = np.asarray(Wq, np.float32)
    Wk = np.asarray(Wk, np.float32)
    Wv = np.asarray(Wv, np.float32)
    Wo = np.asarray(Wo, np.float32)
    cos = np.asarray(cos, np.float32)[0]      # [L, H, 64]
    sin = np.asarray(sin, np.float32)[0]
    qp = np.asarray(q_param, np.float32).reshape(H)
    ls = np.asarray(log_scale, np.float32).reshape(H)
    mask = np.asarray(mask)

    p64 = np.arange(128) % 64

    PM = np.zeros((128, 128), np.float32)
    for dp in range(128):
        base, r = (dp // 64) * 64, dp % 64
        if r < 32:
            PM[base + r + 32, dp] = -1.0
        else:
            PM[base + r - 32, dp] = 1.0
    SW = np.zeros((128, 128), np.float32)
    for dp in range(128):
        SW[(dp + 64) % 128, dp] = 1.0
    ONES = np.ones((128, 128), ml_dtypes.bfloat16)
    IDENT = np.eye(128, dtype=ml_dtypes.bfloat16)

    in_maps = []
    for core in range(8):
        b, g2 = core // 2, core % 2
        heads = list(range(g2 * NH, (g2 + 1) * NH))
        kvs = list(range(g2 * NKV, (g2 + 1) * NKV))

        xb = x[b]
        xh = xb.astype(ml_dtypes.bfloat16)
        xlo = (xb - xh.astype(np.float32)).astype(ml_dtypes.bfloat16)

        wq_c = Wq[:, g2 * NH * 128:(g2 + 1) * NH * 128]
        wk_c = Wk[:, g2 * NKV * 128:(g2 + 1) * NKV * 128]
        wv_c = Wv[:, g2 * NKV * 128:(g2 + 1) * NKV * 128]
        wo_c = Wo[g2 * NH * 128:(g2 + 1) * NH * 128, :]

        wq_p = wq_c.reshape(16, 128, NH * 128).transpose(1, 0, 2).copy()
        wk_p = wk_c.reshape(16, 128, NKV * 128).transpose(1, 0, 2).copy()
        wv_p = wv_c.reshape(16, 128, NKV * 128).transpose(1, 0, 2).copy()
        wo_p = wo_c.reshape(NH, 128, D).transpose(1, 0, 2).astype(ml_dtypes.bfloat16)

        bf = ml_dtypes.bfloat16
        cosq_p = np.ascontiguousarray(
            cos[:, heads, :][:, :, p64].transpose(2, 1, 0)).astype(bf)
        sinq_p = np.ascontiguousarray(
            sin[:, heads, :][:, :, p64].transpose(2, 1, 0)).astype(bf)
        cosk_p = np.ascontiguousarray(
            cos[:, kvs, :][:, :, p64].transpose(2, 1, 0)).astype(bf)
        sink_p = np.ascontiguousarray(
            sin[:, kvs, :][:, :, p64].transpose(2, 1, 0)).astype(bf)

        mb = np.where(mask[b].reshape(NB, 128).T.astype(bool), 0.0, -1e9)
        mb = mb.astype(np.float32)

        cpr = np.tile((-2.0 * np.tanh(qp[heads]))[None, :], (128, 1))
        alp = np.tile((np.exp(ls[heads]) / HD)[None, :], (128, 1))

        in_maps.append({
            "xh": xh, "xl": xlo,
            "wq": wq_p.astype(np.float32), "wk": wk_p.astype(np.float32),
            "wv": wv_p.astype(np.float32), "wo": wo_p,
            "cosq": cosq_p, "sinq": sinq_p, "cosk": cosk_p, "sink": sink_p,
            "maskb": mb, "cprime": cpr.astype(np.float32),
            "alpha": alp.astype(np.float32),
            "pmrot": PM.astype(ml_dtypes.bfloat16),
            "pmswap": SW.astype(ml_dtypes.bfloat16),
            "onesb": ONES, "identb": IDENT,
        })
    return in_maps


def kernel(**inputs):
    if "nc" not in _CACHED:
        _CACHED["nc"] = build_program()
    nc = _CACHED["nc"]
    in_maps = _host_prep(**inputs)
    res = run_bass_kernel_spmd(nc, in_maps, list(range(8))).results
    out = np.empty((B, L, D), np.float32)
    for b in range(B):
        out[b] = (res[2 * b]["y"].astype(np.float32)
                  + res[2 * b + 1]["y"].astype(np.float32))
    return out
